# revision 1
# baseline (speedup 1.0000x reference)
"""Trainium2 Bass kernel for nn_EnhancedBaselineWithReturnBoost.

4-layer transformer encoder (D=256, H=8, DI=1024) over [B=256, S=128] location
sequences, final-token head into V=50000 logits, plus a scatter-add "return
boost" on recent locations, ensembled with sigmoid(ensemble_weight).

Sharding: pure data-parallel over batch across 8 NeuronCores (32 batch items
per core).  One batch item = one 128-token tile (S=128); tiles are processed
in pairs so most matmuls/vector ops run with a 256-wide moving dimension.
All activations stay resident in SBUF; matmuls run in bf16 with fp32 PSUM
accumulation; the residual stream is fp32.

Engine placement: PE does matmuls/transposes + softmax denominators (ones
matmul broadcast straight into the attention-output layout); ACT does exp
(and rsqrt as exp(-0.5*ln(v+eps)) so every ACT func lives in one LUT set —
no steady-state activation-table reloads); DVE does LN stats/apply, ReLU
(+bias), residual adds, normalization; GPSIMD runs the indirect-DMA gathers
and the boost read-modify-write scatter on the logits in HBM.

Host-side preprocessing (exact, standard inference folds): LN gains folded
into adjacent weights (diag(g) @ W), LN biases folded into following bias
vectors (b @ W), bf16 weight casts, posenc table, and per-core boost
(block, offset) one-hot rows for the scatter.  The boost scale
sigmoid(w)*return_strength is computed on device.
"""
import numpy as np
import ml_dtypes
from contextlib import ExitStack

import concourse.bass as bass
import concourse.mybir as mybir
import concourse.tile as tile
from concourse import bacc, bass_utils
from concourse.bass import IndirectOffsetOnAxis
from concourse.masks import make_identity

F32 = mybir.dt.float32
BF16 = mybir.dt.bfloat16
I32 = mybir.dt.int32
AF = mybir.ActivationFunctionType
ALU = mybir.AluOpType

# problem dims (hardcoded per spec)
V, U, D, DI, L, H, B, S = 50000, 1024, 256, 1024, 4, 8, 256, 128
DK = D // H            # 32
NB = 5                 # boost positions
NCORES = 8
BPC = B // NCORES      # 32 batch items per core
P = 128
KD = D // P            # 2 contraction chunks for D
KI = DI // P           # 8 chunks for DI
NV = 500               # head logits chunk width
NVC = V // NV          # 100 chunks
WSLAB = 4              # head chunks per streamed weight tile
BLK = 256              # boost scatter block (f32 elems) = 1KB
NBLK = BPC * V // BLK  # 6250 blocks per core
BCAP = 256             # max boost rows per core (>= BPC*NB=160), 2 rounds of 128
ATT_SCALE = 1.0 / np.sqrt(DK)
EPS = 1e-5


def _posenc():
    pos = np.arange(S)[:, None]
    i = np.arange(0, D, 2)[None, :]
    ang = pos / np.power(10000.0, i / D)
    pe = np.zeros((S, D), dtype=np.float32)
    pe[:, 0::2] = np.sin(ang)
    pe[:, 1::2] = np.cos(ang)
    return pe


def _build(with_ln1_bias: bool, with_bout: bool, reps: int = 1):
    """Build + compile the per-core Bass program (SPMD: same NEFF, per-core data)."""
    nc = bacc.Bacc("TRN2", target_bir_lowering=False, debug=False, num_devices=NCORES)

    # ---- DRAM I/O -----------------------------------------------------------
    locst_d = nc.dram_tensor("locst", [P, BPC], I32, kind="ExternalInput")
    userst_d = nc.dram_tensor("userst", [P, BPC], I32, kind="ExternalInput")
    lemb_d = nc.dram_tensor("lemb", [V, D], F32, kind="ExternalInput")
    uemb_d = nc.dram_tensor("uemb", [U, D], BF16, kind="ExternalInput")
    pos_d = nc.dram_tensor("posenc", [S, D], F32, kind="ExternalInput")
    wq_d = nc.dram_tensor("wq", [L, D, D], BF16, kind="ExternalInput")
    wk_d = nc.dram_tensor("wk", [L, D, D], BF16, kind="ExternalInput")
    wv_d = nc.dram_tensor("wv", [L, D, D], BF16, kind="ExternalInput")
    wo_d = nc.dram_tensor("wo", [L, D, D], BF16, kind="ExternalInput")
    w1_d = nc.dram_tensor("w1", [L, D, DI], BF16, kind="ExternalInput")
    w2_d = nc.dram_tensor("w2", [L, DI, D], BF16, kind="ExternalInput")
    b1t_d = nc.dram_tensor("b1t", [P, L * KI], F32, kind="ExternalInput")
    b2r_d = nc.dram_tensor("b2r", [1, L * D], BF16, kind="ExternalInput")
    lnb_d = nc.dram_tensor("lnbrows", [3, L, 1, D], BF16, kind="ExternalInput")
    wout_d = nc.dram_tensor("wout", [D, V], BF16, kind="ExternalInput")
    bout_d = nc.dram_tensor("bout", [1, V], BF16, kind="ExternalInput")
    ens_d = nc.dram_tensor("ens", [1, 1], F32, kind="ExternalInput")
    rstr_d = nc.dram_tensor("rstr", [1, 1], F32, kind="ExternalInput")
    bidx_d = nc.dram_tensor("bidx", [BCAP, 1], I32, kind="ExternalInput")
    brows_d = nc.dram_tensor("brows", [BCAP, BLK], F32, kind="ExternalInput")
    out_d = nc.dram_tensor("out", [BPC, V], F32, kind="ExternalOutput")

    head_dma_insts = []

    with tile.TileContext(nc) as tc, ExitStack() as ctx:
        cp = ctx.enter_context(tc.tile_pool(name="const", bufs=1))
        wp = ctx.enter_context(tc.tile_pool(name="wts", bufs=1))
        sp = ctx.enter_context(tc.tile_pool(name="work", bufs=3))
        ap_ = ctx.enter_context(tc.tile_pool(name="attw", bufs=3))
        gp = ctx.enter_context(tc.tile_pool(name="gath", bufs=4))
        hp = ctx.enter_context(tc.tile_pool(name="head", bufs=2))
        # PSUM: 8 banks: psA 4x[128,256]f32 + psB 2x[128,512]f32 + psC 2
        psA = ctx.enter_context(tc.tile_pool(name="psA", bufs=4, space="PSUM"))
        psB = ctx.enter_context(tc.tile_pool(name="psB", bufs=2, space="PSUM"))
        psC = ctx.enter_context(tc.tile_pool(name="psC", bufs=2, space="PSUM"))

        def p256(dtype=F32):
            return psA.tile([P, 256], dtype, tag="ps256", name="p256")

        def p512(dtype=F32):
            return psB.tile([P, 512], dtype, tag="ps512", name="p512")

        def pmisc(shape, dtype=F32):
            return psC.tile(list(shape), dtype, tag="misc", name="pmisc")

        # ---- constants ------------------------------------------------------
        ident = cp.tile([P, P], BF16)
        make_identity(nc, ident[:])
        ones_r32 = cp.tile([1, P], F32)
        nc.vector.memset(ones_r32[:], 1.0)
        ones_rbf = cp.tile([1, 256], BF16)
        nc.vector.memset(ones_rbf[:], 1.0)
        ones_m32 = cp.tile([P, DK], BF16)
        nc.vector.memset(ones_m32[:], 1.0)
        eps_c = cp.tile([P, 1], F32)
        nc.vector.memset(eps_c[:], EPS)

        pos_sb = cp.tile([P, D], F32)
        nc.sync.dma_start(out=pos_sb[:], in_=pos_d[:])
        b1t_sb = cp.tile([P, L * KI], F32)
        nc.sync.dma_start(out=b1t_sb[:], in_=b1t_d[:])
        b2r_sb = cp.tile([1, L * D], BF16)
        nc.sync.dma_start(out=b2r_sb[:], in_=b2r_d[:])
        locst_sb = cp.tile([P, BPC], I32)
        nc.sync.dma_start(out=locst_sb[:], in_=locst_d[:])
        userst_sb = cp.tile([P, BPC], I32)
        nc.sync.dma_start(out=userst_sb[:], in_=userst_d[:])
        lnb_sb = None
        if with_ln1_bias:
            lnb_sb = cp.tile([1, 3 * L * D], BF16)
            for t in range(3):
                for l in range(L):
                    nc.sync.dma_start(
                        out=lnb_sb[:, (t * L + l) * D:(t * L + l + 1) * D],
                        in_=lnb_d[t, l],
                    )

        # ---- weights resident in SBUF --------------------------------------
        wq_sb, wk_sb, wv_sb, wo_sb, w1_sb, w2_sb = [], [], [], [], [], []
        for l in range(L):
            for (nm, lst, dram, width) in (
                ("wq", wq_sb, wq_d, D), ("wk", wk_sb, wk_d, D),
                ("wv", wv_sb, wv_d, D), ("wo", wo_sb, wo_d, D),
            ):
                t = wp.tile([P, KD * width], BF16, tag=f"{nm}_{l}", name=f"{nm}_{l}")
                for kc in range(KD):
                    nc.sync.dma_start(
                        out=t[:, kc * width:(kc + 1) * width],
                        in_=dram[l, kc * P:(kc + 1) * P, :],
                    )
                lst.append(t)
            t = wp.tile([P, KD * DI], BF16, tag=f"w1_{l}", name=f"w1_{l}")
            for kc in range(KD):
                nc.sync.dma_start(out=t[:, kc * DI:(kc + 1) * DI],
                                  in_=w1_d[l, kc * P:(kc + 1) * P, :])
            w1_sb.append(t)
            t = wp.tile([P, KI * D], BF16, tag=f"w2_{l}", name=f"w2_{l}")
            for ki in range(KI):
                nc.sync.dma_start(out=t[:, ki * D:(ki + 1) * D],
                                  in_=w2_d[l, ki * P:(ki + 1) * P, :])
            w2_sb.append(t)

        # ---- scalars: w = sigmoid(ens); 1-w ; s = w*rstr --------------------
        ens_sb = cp.tile([1, 1], F32)
        nc.sync.dma_start(out=ens_sb[:], in_=ens_d[:])
        rstr_sb = cp.tile([1, 1], F32)
        nc.sync.dma_start(out=rstr_sb[:], in_=rstr_d[:])
        w_sb = cp.tile([1, 1], F32)
        nc.scalar.activation(out=w_sb[:], in_=ens_sb[:], func=AF.Sigmoid)
        onem_sb = cp.tile([1, 1], F32)
        nc.vector.tensor_scalar(out=onem_sb[:], in0=w_sb[:], scalar1=-1.0,
                                scalar2=1.0, op0=ALU.mult, op1=ALU.add)
        s_sb = cp.tile([1, 1], F32)
        nc.vector.tensor_tensor(out=s_sb[:], in0=w_sb[:], in1=rstr_sb[:], op=ALU.mult)
        ps_a = pmisc([BPC, 1])
        nc.tensor.matmul(out=ps_a[:], lhsT=ones_r32[:, 0:BPC], rhs=onem_sb[:],
                         start=True, stop=True)
        onem32 = cp.tile([BPC, 1], F32)
        nc.scalar.copy(out=onem32[:], in_=ps_a[:])
        ps_b = pmisc([P, 1])
        nc.tensor.matmul(out=ps_b[:], lhsT=ones_r32[:], rhs=s_sb[:],
                         start=True, stop=True)
        sbc = cp.tile([P, 1], F32)
        nc.scalar.copy(out=sbc[:], in_=ps_b[:])
        if with_bout:
            onemw_row32 = cp.tile([1, BPC], F32)
            nc.vector.tensor_scalar(out=onemw_row32[:], in0=ones_r32[:, 0:BPC],
                                    scalar1=onem_sb[:, 0:1], scalar2=None,
                                    op0=ALU.mult)
            onemw_row = cp.tile([1, BPC], BF16)
            nc.vector.tensor_copy(out=onemw_row[:], in_=onemw_row32[:])

        # ---- residual stream ------------------------------------------------
        x_big = cp.tile([P, BPC * D], F32)   # x for all 32 batch tiles

        def ln_pair_stats(xpair):
            """v1-style per-tile LN stats; returns ([rs0,rs1], [nmu0,nmu1])."""
            rs_l, nmu_l = [], []
            for j in range(2):
                st = sp.tile([P, 6], F32, tag=f"lnst{j}", name=f"lnst{j}")
                nc.vector.bn_stats(out=st[:], in_=xpair[:, j * D:(j + 1) * D])
                ag = sp.tile([P, 2], F32, tag=f"lnag{j}", name=f"lnag{j}")
                nc.vector.bn_aggr(out=ag[:], in_=st[:])
                rec = sp.tile([P, 1], F32, tag=f"lnrec{j}", name=f"lnrec{j}")
                nc.scalar.activation(out=rec[:], in_=ag[:, 1:2], func=AF.Ln,
                                     bias=eps_c[:])
                rs = sp.tile([P, 1], F32, tag=f"lnrs{j}", name=f"lnrs{j}")
                nc.scalar.activation(out=rs[:], in_=rec[:], func=AF.Exp, scale=-0.5)
                nmu = sp.tile([P, 1], F32, tag=f"lnnmu{j}", name=f"lnnmu{j}")
                nc.vector.scalar_tensor_tensor(out=nmu[:], in0=ag[:, 0:1], scalar=-1.0,
                                               in1=rs[:], op0=ALU.mult, op1=ALU.mult)
                rs_l.append(rs)
                nmu_l.append(nmu)
            return rs_l, nmu_l

        def transpose_pair(src_bf, dst_bf, eng_off=0):
            """src [128, 512] = two 256-wide tiles (j=0,1).  dst blocks are
            kc-major: dst block (kc*2+j) = rows kc*128.. of tile j's transpose,
            so dst[:, kc*256:(kc+1)*256] is the kc-chunk rhs for the pair."""
            for c in range(4):
                j, kc = c // 2, c % 2
                pt = psA.tile([P, P], BF16, tag="ps256", name="pt")
                nc.tensor.transpose(out=pt[:], in_=src_bf[:, c * P:(c + 1) * P],
                                    identity=ident[:])
                dblk = kc * 2 + j
                if (c + eng_off) % 2 == 0:
                    nc.scalar.copy(out=dst_bf[:, dblk * P:(dblk + 1) * P], in_=pt[:])
                else:
                    nc.vector.tensor_copy(out=dst_bf[:, dblk * P:(dblk + 1) * P],
                                          in_=pt[:])

        for _rep in range(reps):
            head_dma_insts = []
            # ================= per-batch-pair pipeline ==========================
            for bp in range(BPC // 2):
                b0 = 2 * bp
                xpair = x_big[:, b0 * D:(b0 + 2) * D]       # [128, 512]
                for j in range(2):
                    b = b0 + j
                    xb = x_big[:, b * D:(b + 1) * D]
                    xg = gp.tile([P, D], F32, tag="xg", name="xg")
                    nc.gpsimd.indirect_dma_start(
                        out=xg[:], out_offset=None, in_=lemb_d[:],
                        in_offset=IndirectOffsetOnAxis(ap=locst_sb[:, b:b + 1], axis=0))
                    ub = gp.tile([P, D], BF16, tag="ub", name="ub")
                    nc.gpsimd.indirect_dma_start(
                        out=ub[:], out_offset=None, in_=uemb_d[:],
                        in_offset=IndirectOffsetOnAxis(ap=userst_sb[:, b:b + 1], axis=0))
                    nc.vector.tensor_add(out=xb, in0=xg[:], in1=pos_sb[:])
                    nc.vector.tensor_add(out=xb, in0=xb, in1=ub[:])

                for l in range(L):
                    # ---------- LN1 + h1 / h1T (pairwise) ----------
                    rs, nmu = ln_pair_stats(xpair)
                    h1 = sp.tile([P, 2 * D], BF16, tag="h1", name="h1")
                    for j in range(2):
                        nc.scalar.activation(
                            out=h1[:, j * D:(j + 1) * D],
                            in_=x_big[:, (b0 + j) * D:(b0 + j + 1) * D],
                            func=AF.Identity, bias=nmu[j][:], scale=rs[j][:])
                    h1T = sp.tile([P, 4 * P], BF16, tag="h1T", name="h1T")
                    transpose_pair(h1[:], h1T[:])

                    # ---------- Q,K projections (N=256 over the pair) ----------
                    qT = sp.tile([P, 2 * D], BF16, tag="qT", name="qT")
                    kT = sp.tile([P, 2 * D], BF16, tag="kT", name="kT")
                    # layout: [:, m*256 + j*128] = rows m*128.. of tile j's qT/kT
                    for (pi, dst, wsb) in ((0, qT, wq_sb[l]), (1, kT, wk_sb[l])):
                        for m in range(KD):
                            pq = p256(F32)
                            for kc in range(KD):
                                nc.tensor.matmul(
                                    out=pq[:],
                                    lhsT=wsb[:, kc * D + m * P: kc * D + (m + 1) * P],
                                    rhs=h1T[:, kc * 256:(kc + 1) * 256],
                                    start=(kc == 0),
                                    stop=(kc == KD - 1 and lnb_sb is None))
                            if lnb_sb is not None:
                                nc.tensor.matmul(
                                    out=pq[:],
                                    lhsT=lnb_sb[:, (pi * L + l) * D + m * P:
                                                (pi * L + l) * D + (m + 1) * P],
                                    rhs=ones_rbf[:], start=False, stop=True)
                            if (m + pi) % 2 == 0:
                                nc.scalar.copy(out=dst[:, m * 256:(m + 1) * 256], in_=pq[:])
                            else:
                                nc.vector.tensor_copy(out=dst[:, m * 256:(m + 1) * 256],
                                                      in_=pq[:])

                    # ---------- V (per tile, M=tok) ----------
                    v_sbs = []
                    for j in range(2):
                        pv = p256(F32)
                        for kc in range(KD):
                            nc.tensor.matmul(
                                out=pv[:],
                                lhsT=h1T[:, (kc * 2 + j) * P:(kc * 2 + j + 1) * P],
                                rhs=wv_sb[l][:, kc * D:(kc + 1) * D],
                                start=(kc == 0),
                                stop=(kc == KD - 1 and lnb_sb is None))
                        if lnb_sb is not None:
                            nc.tensor.matmul(
                                out=pv[:], lhsT=ones_rbf[:, 0:P],
                                rhs=lnb_sb[:, (2 * L + l) * D:(2 * L + l + 1) * D],
                                start=False, stop=True)
                        v_sb = sp.tile([P, D], BF16, tag=f"vsb{j}", name=f"vsb{j}")
                        if j == 0:
                            nc.scalar.copy(out=v_sb[:], in_=pv[:])
                        else:
                            nc.vector.tensor_copy(out=v_sb[:], in_=pv[:])
                        v_sbs.append(v_sb)

                    # ---------- attention (per tile; exp grouped 4 heads) -------
                    for j in range(2):
                        den_ps = p256(F32)
                        oT_ps = p256(F32)
                        for h in range(H):
                            mq = h // 4
                            po = (h % 4) * DK
                            qs = qT[po:po + DK,
                                    mq * 256 + j * P: mq * 256 + (j + 1) * P]
                            ks = kT[po:po + DK,
                                    mq * 256 + j * P: mq * 256 + (j + 1) * P]
                            ps_s = psA.tile([P, P], F32, tag="ps256", name="ps_s")
                            nc.tensor.matmul(out=ps_s[:], lhsT=ks, rhs=qs,
                                             start=True, stop=True,
                                             tile_position=(po, 0))
                            att1 = ap_.tile([P, P], BF16, tag="att4", name="att1")
                            nc.scalar.activation(out=att1[:], in_=ps_s[:],
                                                 func=AF.Exp, scale=ATT_SCALE)
                            nc.tensor.matmul(
                                out=den_ps[po:po + DK, mq * P:(mq + 1) * P],
                                lhsT=ones_m32[:], rhs=att1[:], start=True, stop=True,
                                tile_position=(0, po))
                            nc.tensor.matmul(
                                out=oT_ps[po:po + DK, mq * P:(mq + 1) * P],
                                lhsT=v_sbs[j][:, h * DK:(h + 1) * DK],
                                rhs=att1[:], start=True, stop=True,
                                tile_position=(0, po))

                        rf_sb = sp.tile([P, D], F32, tag="rfsb", name="rfsb")
                        nc.vector.reciprocal(out=rf_sb[:], in_=den_ps[:])
                        oTn = sp.tile([P, D], BF16, tag="oTn", name="oTn")
                        nc.vector.tensor_tensor(out=oTn[:], in0=oT_ps[:], in1=rf_sb[:],
                                                op=ALU.mult)

                        pxd = p256(F32)
                        for kc in range(KD):
                            nc.tensor.matmul(out=pxd[:], lhsT=oTn[:, kc * P:(kc + 1) * P],
                                             rhs=wo_sb[l][:, kc * D:(kc + 1) * D],
                                             start=(kc == 0), stop=(kc == KD - 1))
                        xb = x_big[:, (b0 + j) * D:(b0 + j + 1) * D]
                        nc.vector.tensor_add(out=xb, in0=xb, in1=pxd[:])

                    # ---------- LN2 + FFN ----------
                    rs2, nmu2 = ln_pair_stats(xpair)
                    h2 = sp.tile([P, 2 * D], BF16, tag="h2", name="h2")
                    for j in range(2):
                        nc.scalar.activation(
                            out=h2[:, j * D:(j + 1) * D],
                            in_=x_big[:, (b0 + j) * D:(b0 + j + 1) * D],
                            func=AF.Identity, bias=nmu2[j][:], scale=rs2[j][:])
                    h2T = sp.tile([P, 4 * P], BF16, tag="h2T", name="h2T")
                    transpose_pair(h2[:], h2T[:], eng_off=1)

                    a_sb = sp.tile([P, 2 * DI], BF16, tag="asb", name="asb")
                    # layout: [:, mi*256 + j*128] = aT rows mi*128.. for tile j
                    for mi in range(KI):
                        pa = p256(F32)
                        for kc in range(KD):
                            nc.tensor.matmul(
                                out=pa[:],
                                lhsT=w1_sb[l][:, kc * DI + mi * P: kc * DI + (mi + 1) * P],
                                rhs=h2T[:, kc * 256:(kc + 1) * 256],
                                start=(kc == 0), stop=(kc == KD - 1))
                        nc.vector.tensor_scalar(
                            out=a_sb[:, mi * 256:(mi + 1) * 256], in0=pa[:],
                            scalar1=b1t_sb[:, l * KI + mi: l * KI + mi + 1],
                            scalar2=0.0, op0=ALU.add, op1=ALU.max)

                    for j in range(2):
                        pxd2 = p256(F32)
                        for ki in range(KI):
                            nc.tensor.matmul(
                                out=pxd2[:],
                                lhsT=a_sb[:, ki * 256 + j * P: ki * 256 + (j + 1) * P],
                                rhs=w2_sb[l][:, ki * D:(ki + 1) * D],
                                start=(ki == 0), stop=False)
                        nc.tensor.matmul(out=pxd2[:], lhsT=ones_rbf[:, 0:P],
                                         rhs=b2r_sb[:, l * D:(l + 1) * D],
                                         start=False, stop=True)
                        xb = x_big[:, (b0 + j) * D:(b0 + j + 1) * D]
                        nc.vector.tensor_add(out=xb, in0=xb, in1=pxd2[:])

            # ================= final LN + head ==================================
            xl = cp.tile([BPC, D], F32)
            for b in range(BPC):
                nc.sync.dma_start(out=xl[b:b + 1, :],
                                  in_=x_big[P - 1:P, b * D:(b + 1) * D])
            stf = cp.tile([BPC, 6], F32)
            nc.vector.bn_stats(out=stf[:], in_=xl[:])
            agf = cp.tile([BPC, 2], F32)
            nc.vector.bn_aggr(out=agf[:], in_=stf[:])
            lnvf = cp.tile([BPC, 1], F32)
            nc.scalar.activation(out=lnvf[:], in_=agf[:, 1:2], func=AF.Ln, bias=eps_c[0:BPC])
            rsf = cp.tile([BPC, 1], F32)
            nc.scalar.activation(out=rsf[:], in_=lnvf[:], func=AF.Exp, scale=-0.5)
            nc.vector.tensor_tensor(out=rsf[:], in0=rsf[:], in1=onem32[:], op=ALU.mult)
            nmuf = cp.tile([BPC, 1], F32)
            nc.vector.scalar_tensor_tensor(out=nmuf[:], in0=agf[:, 0:1], scalar=-1.0,
                                           in1=rsf[:], op0=ALU.mult, op1=ALU.mult)
            xls = cp.tile([BPC, D], BF16)
            nc.scalar.activation(out=xls[:], in_=xl[:], func=AF.Identity,
                                 bias=nmuf[:], scale=rsf[:])
            xlT = cp.tile([P, KD * BPC], BF16)
            for c in range(KD):
                pt2 = pmisc([P, BPC], BF16)
                nc.tensor.transpose(out=pt2[:], in_=xls[:, c * P:(c + 1) * P],
                                    identity=ident[0:BPC, 0:BPC])
                nc.scalar.copy(out=xlT[:, c * BPC:(c + 1) * BPC], in_=pt2[:])

            for ns in range(0, NVC, WSLAB):
                wt = hp.tile([P, KD * WSLAB * NV], BF16, tag="wout", name="wout")
                for kc in range(KD):
                    nc.sync.dma_start(
                        out=wt[:, kc * WSLAB * NV:(kc + 1) * WSLAB * NV],
                        in_=wout_d[kc * P:(kc + 1) * P, ns * NV:(ns + WSLAB) * NV])
                if with_bout:
                    bt = hp.tile([1, WSLAB * NV], BF16, tag="boutt", name="boutt")
                    nc.sync.dma_start(out=bt[:], in_=bout_d[:, ns * NV:(ns + WSLAB) * NV])
                for si in range(WSLAB):
                    n = ns + si
                    plog = pmisc([BPC, NV])
                    for kc in range(KD):
                        nc.tensor.matmul(
                            out=plog[:], lhsT=xlT[:, kc * BPC:(kc + 1) * BPC],
                            rhs=wt[:, kc * WSLAB * NV + si * NV:
                                   kc * WSLAB * NV + (si + 1) * NV],
                            start=(kc == 0), stop=(kc == KD - 1 and not with_bout))
                    if with_bout:
                        nc.tensor.matmul(out=plog[:], lhsT=onemw_row[:],
                                         rhs=bt[:, si * NV:(si + 1) * NV],
                                         start=False, stop=True)
                    lsb = hp.tile([BPC, NV], F32, tag="lsb", name="lsb")
                    if n % 2 == 0:
                        nc.scalar.copy(out=lsb[:], in_=plog[:])
                    else:
                        nc.vector.tensor_copy(out=lsb[:], in_=plog[:])
                    di = nc.sync.dma_start(out=out_d[:, n * NV:(n + 1) * NV], in_=lsb[:])
                    head_dma_insts.append(di.ins)

            # ================= boost RMW scatter-add ============================
            out_flat = out_d[:].rearrange("a v -> (a v)").rearrange("(n c) -> n c", c=BLK)
            for r in range(BCAP // P):
                bi = gp.tile([P, 1], I32, tag="bidx", name="bidx")
                nc.sync.dma_start(out=bi[:], in_=bidx_d[r * P:(r + 1) * P])
                br = gp.tile([P, BLK], F32, tag="brow", name="brow")
                nc.sync.dma_start(out=br[:], in_=brows_d[r * P:(r + 1) * P, :])
                g = gp.tile([P, BLK], F32, tag="grmw", name="grmw")
                nc.vector.memset(g[:], 0.0)
                gi = nc.gpsimd.indirect_dma_start(
                    out=g[:], out_offset=None, in_=out_flat,
                    in_offset=IndirectOffsetOnAxis(ap=bi[:, :1], axis=0),
                    bounds_check=NBLK - 1, oob_is_err=False)
                for di in head_dma_insts:
                    tile.add_dep_helper(gi.ins, di, reason="boost RMW after head DMA")
                nc.vector.scalar_tensor_tensor(out=g[:], in0=br[:], scalar=sbc[:],
                                               in1=g[:], op0=ALU.mult, op1=ALU.add)
                nc.gpsimd.indirect_dma_start(
                    out=out_flat, out_offset=IndirectOffsetOnAxis(ap=bi[:, :1], axis=0),
                    in_=g[:], in_offset=None,
                    bounds_check=NBLK - 1, oob_is_err=False)

    nc.compile()
    return nc


_CACHE = {}


def _get_nc(with_ln1_bias: bool, with_bout: bool, reps: int = 1):
    key = (bool(with_ln1_bias), bool(with_bout), reps)
    if key not in _CACHE:
        _CACHE[key] = _build(with_ln1_bias, with_bout, reps)
    return _CACHE[key]


def _prep_inputs(inputs):
    """Host-side preprocessing: returns (in_maps, with_ln1_bias, with_bout)."""
    f = lambda a: np.asarray(a, dtype=np.float32)
    bf = lambda a: np.ascontiguousarray(a).astype(ml_dtypes.bfloat16)

    locations = np.asarray(inputs["locations"]).astype(np.int64)
    users = np.asarray(inputs["users"]).astype(np.int64)
    loc_emb = f(inputs["loc_emb"])
    user_emb = f(inputs["user_emb"])
    Wq, Wk, Wv, Wo = (f(inputs[k]) for k in ("Wq", "Wk", "Wv", "Wo"))
    W1, W2 = f(inputs["W1"]), f(inputs["W2"])
    b1, b2 = f(inputs["b1"]), f(inputs["b2"])
    ln1_g, ln1_b = f(inputs["ln1_g"]), f(inputs["ln1_b"])
    ln2_g, ln2_b = f(inputs["ln2_g"]), f(inputs["ln2_b"])
    lnf_g, lnf_b = f(inputs["lnf_g"]), f(inputs["lnf_b"])
    W_out, b_out = f(inputs["W_out"]), f(inputs["b_out"])
    position_boost = f(inputs["position_boost"])
    return_strength = f(inputs["return_strength"]).reshape(1, 1)
    ensemble_weight = f(inputs["ensemble_weight"]).reshape(1, 1)

    # LN folds (exact linear algebra, done once on host)
    wq_e = ln1_g[:, :, None] * Wq
    wk_e = ln1_g[:, :, None] * Wk
    wv_e = ln1_g[:, :, None] * Wv
    w1_e = ln2_g[:, :, None] * W1
    b1_e = b1 + np.einsum("ld,ldj->lj", ln2_b, W1)
    wout_e = lnf_g[:, None] * W_out
    bout_e = b_out + lnf_b @ W_out
    qrow = np.einsum("ld,ldj->lj", ln1_b, Wq)
    krow = np.einsum("ld,ldj->lj", ln1_b, Wk)
    vrow = np.einsum("ld,ldj->lj", ln1_b, Wv)
    with_ln1_bias = bool(max(np.abs(qrow).max(), np.abs(krow).max(),
                             np.abs(vrow).max()) > 0)
    with_bout = bool(np.abs(bout_e).max() > 0)
    lnb_rows = np.stack([qrow, krow, vrow])[:, :, None, :]  # [3, L, 1, D]

    b1t = b1_e.reshape(L * KI, P).T.copy()  # [P, L*KI]
    posenc = _posenc()

    shared = {
        "lemb": loc_emb,
        "uemb": bf(user_emb),
        "posenc": posenc,
        "wq": bf(wq_e), "wk": bf(wk_e), "wv": bf(wv_e), "wo": bf(Wo),
        "w1": bf(w1_e), "w2": bf(W2),
        "b1t": b1t, "b2r": bf(b2.reshape(1, L * D)),
        "lnbrows": bf(lnb_rows),
        "wout": bf(wout_e), "bout": bf(bout_e[None, :]),
        "ens": ensemble_weight, "rstr": return_strength,
    }

    in_maps = []
    for c in range(NCORES):
        lc = locations[c * BPC:(c + 1) * BPC]            # [BPC, S]
        uc = users[c * BPC:(c + 1) * BPC]
        rows = {}
        for b in range(BPC):
            for j in range(NB):
                col = int(lc[b, S - 1 - j])
                flat = b * V + col
                blk, off = flat // BLK, flat % BLK
                if blk not in rows:
                    rows[blk] = np.zeros(BLK, np.float32)
                rows[blk][off] += position_boost[j]
        bidx = np.full((BCAP, 1), 1 << 20, np.int32)
        brows = np.zeros((BCAP, BLK), np.float32)
        for i, (blk, row) in enumerate(sorted(rows.items())):
            bidx[i, 0] = blk
            brows[i] = row
        m = dict(shared)
        m["locst"] = np.ascontiguousarray(lc.astype(np.int32).T)       # [S, BPC]
        m["userst"] = np.ascontiguousarray(
            np.repeat(uc.astype(np.int32)[None, :], P, axis=0))        # [P, BPC]
        m["bidx"] = bidx
        m["brows"] = brows
        in_maps.append(m)
    return in_maps, with_ln1_bias, with_bout


def kernel(**inputs) -> np.ndarray:
    in_maps, with_ln1_bias, with_bout = _prep_inputs(inputs)
    nc = _get_nc(with_ln1_bias, with_bout)
    res = bass_utils.run_bass_kernel_spmd(nc, in_maps, core_ids=list(range(NCORES)))
    return np.concatenate([r["out"] for r in res.results], axis=0)



# revision 25
# speedup vs baseline: 6.6817x; 6.6817x over previous
"""Trainium2 Bass kernel for nn_EnhancedBaselineWithReturnBoost.

4-layer transformer encoder (D=256, H=8, DI=1024) over [B=256, S=128] location
sequences, final-token head into V=50000 logits, plus a scatter-add "return
boost" on recent locations, ensembled with sigmoid(ensemble_weight).

Sharding: pure data-parallel over batch across 8 NeuronCores (32 batch items
per core).  One batch item = one 128-token tile (S=128); tiles are processed
in QUADS (4 batch items, 512-wide moving dimension) to amortize instruction
overheads.  The residual stream x is bf16 in SBUF; matmuls run bf16 with fp32
PSUM accumulation.

Key structural choices:
- LN apply + transpose fused into PE: h^T = x^T @ diag(rs) + 1 (x) nmu_row, so
  the normalized, transposed activations come straight out of the tensor
  engine (no separate ACT apply, no separate transpose pass).
- LN stats (sum, sum of squares) computed on the otherwise-idle GPSIMD engine
  via scalar_tensor_tensor accum_out; small [128,4] fixups on DVE/ACT.
- Attention scores for 4 heads share one [128,512] PSUM bank; a single ACT exp
  evacuates each group.  Softmax denominators via ones-matmul (PE) into the
  same layout as O^T; one reciprocal + one multiply per tile.
- Residual adds are folded into the projection matmuls by injecting x into
  PSUM with an identity matmul; the PSUM->SBUF evacuation writes the new x.
- ACT table thrash eliminated by pinning Exp/Ln to the combined
  natural_log_exp_and_others set (see _pinned_act_tables below).
- Head ([D,V] streamed bf16) and the boost RMW scatter as in v1.
"""
import numpy as np
import ml_dtypes
from contextlib import ExitStack

import concourse.bass as bass
import concourse.mybir as mybir
import concourse.tile as tile
from concourse import bacc, bass_utils
from concourse.bass import IndirectOffsetOnAxis
from concourse.masks import make_identity

# ---------------------------------------------------------------------------
# Pin Exp/Ln to the one activation-table set that contains both
# ("natural_log_exp_and_others").  The stock table-load pass picks the FIRST
# set containing each function ("exp_and_others" for Exp, "natural_log" for
# Ln), which forces an ACT table reload (~1.3us) on every Ln->Exp transition
# — 514 reloads/core in the v1 kernel.  Claiming Exp/Ln only in the combined
# set (a valid id in act_info.json that really contains exp+ln+identity+copy)
# makes the pass emit a single load.  Compile-time metadata only; the emitted
# act_func_set_id remains correct for walrus/runtime.
import concourse.bacc as _bacc_mod

_AF = mybir.ActivationFunctionType
_orig_get_act_tables = _bacc_mod.get_activation_tables


def _pinned_act_tables(arch):
    out = {}
    for name, funcs in _orig_get_act_tables(arch).items():
        if name != "natural_log_exp_and_others":
            funcs = set(funcs) - {_AF.Exp, _AF.Ln}
        out[name] = set(funcs)
    return out


_bacc_mod.get_activation_tables = _pinned_act_tables

F32 = mybir.dt.float32
BF16 = mybir.dt.bfloat16
F8 = mybir.dt.float8e4
DR = mybir.MatmulPerfMode.DoubleRow
I32 = mybir.dt.int32
AF = mybir.ActivationFunctionType
ALU = mybir.AluOpType

# problem dims (hardcoded per spec)
V, U, D, DI, L, H, B, S = 50000, 1024, 256, 1024, 4, 8, 256, 128
DK = D // H            # 32
NB = 5                 # boost positions
NCORES = 8
BPC = B // NCORES      # 32 batch items per core
P = 128
KD = D // P            # 2 contraction chunks for D
KI = DI // P           # 8 chunks for DI
QUAD = 4               # batch tiles per processing group
TOK = QUAD * P         # 512-wide moving dimension
NQ = BPC // QUAD       # 8 quads per core
NV = 500               # head logits chunk width
NVC = V // NV          # 100 chunks
WSLAB = 4              # head chunks per streamed weight tile
BLK = 256              # boost scatter block (f32 elems) = 1KB
NBLK = BPC * V // BLK  # 6250 blocks per core
BCAP = 256             # max boost rows per core (>= BPC*NB=160), 2 rounds of 128
ATT_SCALE = 1.0 / np.sqrt(DK)
EPS = 1e-5


def _posenc():
    pos = np.arange(S)[:, None]
    i = np.arange(0, D, 2)[None, :]
    ang = pos / np.power(10000.0, i / D)
    pe = np.zeros((S, D), dtype=np.float32)
    pe[:, 0::2] = np.sin(ang)
    pe[:, 1::2] = np.cos(ang)
    return pe


def _build(with_ln1_bias: bool, with_bout: bool, reps: int = 1):
    """Build + compile the per-core Bass program (SPMD: same NEFF, per-core data)."""
    nc = bacc.Bacc("TRN2", target_bir_lowering=False, debug=False, num_devices=NCORES)

    # ---- DRAM I/O -----------------------------------------------------------
    locst_d = nc.dram_tensor("locst", [P, BPC], I32, kind="ExternalInput")
    userst_d = nc.dram_tensor("userst", [P, BPC], I32, kind="ExternalInput")
    lemb_d = nc.dram_tensor("lemb", [V, D], BF16, kind="ExternalInput")
    uemb_d = nc.dram_tensor("uemb", [U, D], BF16, kind="ExternalInput")
    pos_d = nc.dram_tensor("posenc", [S, D], BF16, kind="ExternalInput")
    wq_d = nc.dram_tensor("wq", [L, D, D], F8, kind="ExternalInput")
    wk_d = nc.dram_tensor("wk", [L, D, D], F8, kind="ExternalInput")
    wv_d = nc.dram_tensor("wv", [L, D, D], F8, kind="ExternalInput")
    wo_d = nc.dram_tensor("wo", [L, D, D], F8, kind="ExternalInput")
    w1_d = nc.dram_tensor("w1", [L, D, DI], F8, kind="ExternalInput")
    w2_d = nc.dram_tensor("w2", [L, DI, D], F8, kind="ExternalInput")
    b1t_d = nc.dram_tensor("b1t", [P, L * KI], F32, kind="ExternalInput")
    b2r_d = nc.dram_tensor("b2r", [1, L * D], BF16, kind="ExternalInput")
    lnb_d = nc.dram_tensor("lnbrows", [3, L, 1, D], BF16, kind="ExternalInput")
    wout_d = nc.dram_tensor("wout", [D, V], F8, kind="ExternalInput")
    bout_d = nc.dram_tensor("bout", [1, V], BF16, kind="ExternalInput")
    onem_d = nc.dram_tensor("onem32", [BPC, 1], F32, kind="ExternalInput")
    sbc_d = nc.dram_tensor("sbcv", [P, 1], F32, kind="ExternalInput")
    onemr_d = nc.dram_tensor("onemw_row", [1, BPC], BF16, kind="ExternalInput")
    bidx_d = nc.dram_tensor("bidx", [BCAP, 1], I32, kind="ExternalInput")
    brows_d = nc.dram_tensor("brows", [BCAP, BLK], F32, kind="ExternalInput")
    out_d = nc.dram_tensor("out", [BPC, V], F32, kind="ExternalOutput")

    head_dma_insts = []

    with tile.TileContext(nc) as tc, ExitStack() as ctx:
        cp = ctx.enter_context(tc.tile_pool(name="const", bufs=1))
        wp = ctx.enter_context(tc.tile_pool(name="wts", bufs=1))
        sp = ctx.enter_context(tc.tile_pool(name="work", bufs=2))
        ap_ = ctx.enter_context(tc.tile_pool(name="attw", bufs=3))
        gp = ctx.enter_context(tc.tile_pool(name="gath", bufs=4))
        hp = ctx.enter_context(tc.tile_pool(name="head", bufs=2))
        # PSUM: 8 banks = psP 3x[128,512] + psS 2 (scores) + psD 3 (den/oT 2
        # + misc 1).  PSUM-bank rule (HW-probed): matmuls with different
        # tile_position row groups run concurrently on different sub-arrays
        # and must NOT share a destination bank; same row group serializes
        # and may share.
        psP = ctx.enter_context(tc.tile_pool(name="psP", bufs=3, space="PSUM"))
        psS = ctx.enter_context(tc.tile_pool(name="psS", bufs=2, space="PSUM"))
        psD = ctx.enter_context(tc.tile_pool(name="psD", bufs=2, space="PSUM"))

        def pproj():
            return psP.tile([P, TOK], F32, tag="proj", name="pproj")

        def pscore():
            return psS.tile([P, TOK], F32, tag="score", name="pscore")

        def pmisc(dtype=BF16):
            return psD.tile([P, TOK], dtype, tag="misc", name="pmisc", bufs=1)

        # ---- constants ------------------------------------------------------
        ident = cp.tile([P, P], BF16)
        make_identity(nc, ident[:])
        ones_rbf = cp.tile([1, TOK], BF16)
        nc.vector.memset(ones_rbf[:], 1.0)
        ones_m32 = cp.tile([P, DK], BF16)
        nc.vector.memset(ones_m32[:], 1.0)
        eps_c = cp.tile([P, 1], F32)
        nc.vector.memset(eps_c[:], EPS)

        pos_sb = cp.tile([P, D], BF16)
        nc.sync.dma_start(out=pos_sb[:], in_=pos_d[:])
        b1t_sb = cp.tile([P, L * KI], F32)
        nc.sync.dma_start(out=b1t_sb[:], in_=b1t_d[:])
        b2r_sb = cp.tile([1, L * D], BF16)
        nc.sync.dma_start(out=b2r_sb[:], in_=b2r_d[:])
        locst_sb = cp.tile([P, BPC], I32)
        nc.sync.dma_start(out=locst_sb[:], in_=locst_d[:])
        userst_sb = cp.tile([P, BPC], I32)
        nc.sync.dma_start(out=userst_sb[:], in_=userst_d[:])
        lnb_sb = None
        if with_ln1_bias:
            lnb_sb = cp.tile([1, 3 * L * D], BF16)
            for t in range(3):
                for l in range(L):
                    nc.sync.dma_start(
                        out=lnb_sb[:, (t * L + l) * D:(t * L + l + 1) * D],
                        in_=lnb_d[t, l],
                    )

        # ---- weights resident in SBUF --------------------------------------
        wq_sb, wk_sb, wv_sb, wo_sb, w1_sb, w2_sb = [], [], [], [], [], []
        for l in range(L):
            for (nm, lst, dram, width) in (
                ("wq", wq_sb, wq_d, D), ("wk", wk_sb, wk_d, D),
                ("wv", wv_sb, wv_d, D), ("wo", wo_sb, wo_d, D),
            ):
                t = wp.tile([P, KD * width], F8, tag=f"{nm}_{l}", name=f"{nm}_{l}")
                for kc in range(KD):
                    nc.sync.dma_start(
                        out=t[:, kc * width:(kc + 1) * width],
                        in_=dram[l, kc * P:(kc + 1) * P, :],
                    )
                lst.append(t)
            t = wp.tile([P, KD * DI], F8, tag=f"w1_{l}", name=f"w1_{l}")
            for kc in range(KD):
                nc.sync.dma_start(out=t[:, kc * DI:(kc + 1) * DI],
                                  in_=w1_d[l, kc * P:(kc + 1) * P, :])
            w1_sb.append(t)
            t = wp.tile([P, KI * D], F8, tag=f"w2_{l}", name=f"w2_{l}")
            for ki in range(KI):
                nc.sync.dma_start(out=t[:, ki * D:(ki + 1) * D],
                                  in_=w2_d[l, ki * P:(ki + 1) * P, :])
            w2_sb.append(t)

        # ---- ensemble scalars (computed host-side, DMA'd in) ----------------
        onem32 = cp.tile([BPC, 1], F32)
        nc.sync.dma_start(out=onem32[:], in_=onem_d[:])
        sbc = cp.tile([P, 1], F32)
        nc.sync.dma_start(out=sbc[:], in_=sbc_d[:])
        if with_bout:
            onemw_row = cp.tile([1, BPC], BF16)
            nc.sync.dma_start(out=onemw_row[:], in_=onemr_d[:])

        # ---- residual stream ------------------------------------------------
        x_big = cp.tile([P, BPC * D], BF16)   # x for all 32 batch tiles

        def ln_to_hT(b0, which):
            """LN of x tiles b0..b0+3 fused into transposed output:
            hT[:, kc*TOK + t*P + tok] = ((x[tok, kc*128+f] - mu) * rs).

            Stats on GPSIMD (accum_out), fixups on DVE, Ln/Exp on ACT,
            h^T = x^T @ diag(rs) + ones^T (x) nmu_row on PE."""
            agq = sp.tile([P, 2 * QUAD], F32, tag=f"lnag{which}", name="agq")
            for t in range(QUAD):
                xt = x_big[:, (b0 + t) * D:(b0 + t + 1) * D]
                st = sp.tile([P, 6], F32, tag="lnst", name="st", bufs=4)
                nc.vector.bn_stats(out=st[:], in_=xt)
                nc.vector.bn_aggr(out=agq[:, 2 * t:2 * t + 2], in_=st[:])
            lnv = sp.tile([P, QUAD], F32, tag=f"lnlnv{which}", name="lnv")
            nc.scalar.activation(out=lnv[:], in_=agq[:, 1:2 * QUAD:2], func=AF.Ln,
                                 bias=eps_c[:])
            rs_q = sp.tile([P, QUAD], F32, tag=f"lnrs{which}", name="rs_q")
            nc.scalar.activation(out=rs_q[:], in_=lnv[:], func=AF.Exp, scale=-0.5)
            nmu_q = sp.tile([P, QUAD], BF16, tag=f"lnnmu{which}", name="nmu_q")
            nc.vector.scalar_tensor_tensor(out=nmu_q[:], in0=agq[:, 0:2 * QUAD:2],
                                           scalar=-1.0, in1=rs_q[:],
                                           op0=ALU.mult, op1=ALU.mult)
            # nmu as a row [1, TOK]: per-tile PE transpose of [128,1] -> [1,128]
            nmt_ps = pmisc()
            for t in range(QUAD):
                nc.tensor.transpose(out=nmt_ps[0:1, t * P:(t + 1) * P],
                                    in_=nmu_q[:, t:t + 1], identity=ident[:])
            nmursT = sp.tile([1, TOK], BF16, tag=f"lnrow{which}", name="nmursT")
            nc.vector.tensor_copy(out=nmursT[:], in_=nmt_ps[0:1, :])

            diags = []
            for t in range(QUAD):
                dg = sp.tile([P, P], BF16, tag="lndiag", name="dg", bufs=8)
                nc.vector.tensor_scalar(out=dg[:], in0=ident[:],
                                        scalar1=rs_q[:, t:t + 1], scalar2=None,
                                        op0=ALU.mult)
                diags.append(dg)

            hT = sp.tile([P, KD * TOK], F8, tag=f"hT{which}", name="hT")
            hTv = hT[:].rearrange("p (kc t tok) -> p kc t tok", kc=KD, t=QUAD, tok=P)
            for u in range(2):
                hps = pproj()
                for ti in range(2):
                    t = 2 * u + ti
                    xt = x_big[:, (b0 + t) * D:(b0 + t + 1) * D]
                    for kc in range(KD):
                        off = ti * 2 * P + kc * P
                        nc.tensor.matmul(
                            out=hps[:, off:off + P],
                            lhsT=xt[:, kc * P:(kc + 1) * P], rhs=diags[t][:],
                            start=(ti == 0 and kc == 0), stop=False)
                        nc.tensor.matmul(
                            out=hps[:, off:off + P],
                            lhsT=ones_rbf[:, 0:P], rhs=nmursT[:, t * P:(t + 1) * P],
                            start=False, stop=(ti == 1 and kc == KD - 1))
                # evac: bank layout (ti, kc, tok) -> hT layout (kc, t, tok)
                dst = hTv[:, :, 2 * u:2 * u + 2, :].rearrange(
                    "p kc t tok -> p t kc tok")
                src = hps[:].rearrange("p (t kc tok) -> p t kc tok",
                                       t=2, kc=KD, tok=P)
                if u == 0:
                    nc.scalar.copy(out=dst, in_=src)
                else:
                    nc.vector.tensor_copy(out=dst, in_=src)
            return hT

        for _rep in range(reps):
            head_dma_insts = []
            # ================= per-quad pipeline ================================
            for q in range(NQ):
                b0 = QUAD * q
                # ---------- embeddings ----------
                for t in range(QUAD):
                    b = b0 + t
                    xt = x_big[:, b * D:(b + 1) * D]
                    xg = gp.tile([P, D], BF16, tag="xg", name="xg")
                    nc.gpsimd.indirect_dma_start(
                        out=xg[:], out_offset=None, in_=lemb_d[:],
                        in_offset=IndirectOffsetOnAxis(ap=locst_sb[:, b:b + 1], axis=0))
                    ub = gp.tile([P, D], BF16, tag="ub", name="ub")
                    nc.gpsimd.indirect_dma_start(
                        out=ub[:], out_offset=None, in_=uemb_d[:],
                        in_offset=IndirectOffsetOnAxis(ap=userst_sb[:, b:b + 1], axis=0))
                    nc.vector.tensor_add(out=xt, in0=xg[:], in1=pos_sb[:])
                    nc.vector.tensor_add(out=xt, in0=xt, in1=ub[:])

                for l in range(L):
                    # ---------- LN1 -> h1T ----------
                    h1T = ln_to_hT(b0, 1)

                    # ---------- Q,K projections (feature-major, N=TOK) -------
                    qT = sp.tile([P, KD * TOK], BF16, tag="qT", name="qT")
                    kT = sp.tile([P, KD * TOK], BF16, tag="kT", name="kT")
                    h1T3 = h1T[:].rearrange("p (kc tok) -> p kc tok", kc=KD)
                    for pi, (dst, wsb) in enumerate(((qT, wq_sb[l]), (kT, wk_sb[l]))):
                        w3 = wsb[:].rearrange("p (kc d) -> p kc d", kc=KD)
                        for m in range(KD):
                            pq = pproj()
                            nc.tensor.matmul(
                                out=pq[:], lhsT=w3[:, :, m * P:(m + 1) * P],
                                rhs=h1T3, perf_mode=DR,
                                start=True, stop=(lnb_sb is None))
                            if lnb_sb is not None:
                                nc.tensor.matmul(
                                    out=pq[:],
                                    lhsT=lnb_sb[:, (pi * L + l) * D + m * P:
                                                (pi * L + l) * D + (m + 1) * P],
                                    rhs=ones_rbf[:], start=False, stop=True)
                            if (m + pi) % 2 == 0:
                                nc.scalar.copy(out=dst[:, m * TOK:(m + 1) * TOK],
                                               in_=pq[:])
                            else:
                                nc.vector.tensor_copy(
                                    out=dst[:, m * TOK:(m + 1) * TOK], in_=pq[:])

                    # ---------- V (token-major per tile) ----------
                    v_sb = sp.tile([P, QUAD * D], BF16, tag="vsb", name="v_sb")
                    wv3 = wv_sb[l][:].rearrange("p (kc d) -> p kc d", kc=KD)
                    for u in range(2):
                        pv = pproj()
                        for ti in range(2):
                            t = 2 * u + ti
                            nc.tensor.matmul(
                                out=pv[:, ti * D:(ti + 1) * D],
                                lhsT=h1T3[:, :, t * P:(t + 1) * P],
                                rhs=wv3, perf_mode=DR,
                                start=(ti == 0),
                                stop=(ti == 1 and lnb_sb is None))
                            if lnb_sb is not None:
                                nc.tensor.matmul(
                                    out=pv[:, ti * D:(ti + 1) * D],
                                    lhsT=ones_rbf[:, 0:P],
                                    rhs=lnb_sb[:, (2 * L + l) * D:(2 * L + l + 1) * D],
                                    start=False, stop=(ti == 1))
                        if u == 0:
                            nc.scalar.copy(out=v_sb[:, 0:2 * D], in_=pv[:])
                        else:
                            nc.vector.tensor_copy(out=v_sb[:, 2 * D:4 * D], in_=pv[:])

                    # ---------- attention (per tile-pair) ----------
                    # Scores grouped by head row-group h4: the 4 matmuls for
                    # (tp, mq) share tile_position row group (h4*DK, 0), so
                    # they may legally share one PSUM bank (they serialize).
                    # One exp evacuates each h4 bank.
                    oTn_l = []
                    for u in range(2):
                        att = ap_.tile([P, 4 * TOK], BF16, tag="att", name="att",
                                       bufs=2)
                        for h4 in range(4):
                            po = h4 * DK
                            sc = pscore()
                            for tp in range(2):
                                for mq in range(2):
                                    cs = mq * TOK + (2 * u + tp) * P
                                    nc.tensor.matmul(
                                        out=sc[:, (tp * 2 + mq) * P:
                                               (tp * 2 + mq + 1) * P],
                                        lhsT=kT[po:po + DK, cs:cs + P],
                                        rhs=qT[po:po + DK, cs:cs + P],
                                        start=True, stop=True,
                                        tile_position=(po, 0))
                            nc.scalar.activation(
                                out=att[:, h4 * TOK:(h4 + 1) * TOK], in_=sc[:],
                                func=AF.Exp, scale=ATT_SCALE)
                        for tp in range(2):
                            t = 2 * u + tp
                            # den and oT in separate banks so the reciprocal
                            # (den read) can't collide with oT matmul writes.
                            dn = psD.tile([P, D], F32, tag="denoT", name="dn")
                            ot = psD.tile([P, D], F32, tag="denoT", name="ot")
                            for h in range(H):
                                mq, h4 = h // 4, h % 4
                                po = h4 * DK
                                asl = att[:, h4 * TOK + (tp * 2 + mq) * P:
                                          h4 * TOK + (tp * 2 + mq + 1) * P]
                                nc.tensor.matmul(
                                    out=dn[po:po + DK, mq * P:(mq + 1) * P],
                                    lhsT=ones_m32[:], rhs=asl,
                                    start=True, stop=True,
                                    tile_position=(0, po))
                                nc.tensor.matmul(
                                    out=ot[po:po + DK, mq * P:(mq + 1) * P],
                                    lhsT=v_sb[:, t * D + h * DK:
                                              t * D + (h + 1) * DK],
                                    rhs=asl, start=True, stop=True,
                                    tile_position=(0, po))
                            rf = sp.tile([P, D], F32, tag="rf", name="rf", bufs=4)
                            nc.vector.reciprocal(out=rf[:], in_=dn[:])
                            oTn = sp.tile([P, D], F8, tag="oTn", name="oTn",
                                          bufs=4)
                            nc.vector.tensor_tensor(out=oTn[:], in0=ot[:],
                                                    in1=rf[:], op=ALU.mult)
                            oTn_l.append(oTn)

                    # ---------- O projection + residual (x injected) ----------
                    for u in range(2):
                        pox = pproj()
                        for ti in range(2):
                            t = 2 * u + ti
                            xt = x_big[:, (b0 + t) * D:(b0 + t + 1) * D]
                            nc.tensor.matmul(out=pox[:, ti * D:(ti + 1) * D],
                                             lhsT=ident[:], rhs=xt,
                                             start=(ti == 0), stop=False)
                            oTn3 = oTn_l[t][:].rearrange(
                                "p (kc q) -> p kc q", kc=KD)
                            wo3 = wo_sb[l][:].rearrange(
                                "p (kc d) -> p kc d", kc=KD)
                            nc.tensor.matmul(
                                out=pox[:, ti * D:(ti + 1) * D],
                                lhsT=oTn3, rhs=wo3, perf_mode=DR,
                                start=False, stop=(ti == 1))
                        xpair = x_big[:, (b0 + 2 * u) * D:(b0 + 2 * u + 2) * D]
                        if u == 0:
                            nc.scalar.copy(out=xpair, in_=pox[:])
                        else:
                            nc.vector.tensor_copy(out=xpair, in_=pox[:])

                    # ---------- LN2 -> h2T, FFN ----------
                    h2T = ln_to_hT(b0, 2)

                    a_sb = sp.tile([P, KI * TOK], F8, tag="asb", name="a_sb")
                    h2T3 = h2T[:].rearrange("p (kc tok) -> p kc tok", kc=KD)
                    w13 = w1_sb[l][:].rearrange("p (kc d) -> p kc d", kc=KD)
                    for mi in range(KI):
                        pa = pproj()
                        nc.tensor.matmul(
                            out=pa[:], lhsT=w13[:, :, mi * P:(mi + 1) * P],
                            rhs=h2T3, perf_mode=DR, start=True, stop=True)
                        if mi % 2 == 0:
                            nc.scalar.activation(
                                out=a_sb[:, mi * TOK:(mi + 1) * TOK], in_=pa[:],
                                func=AF.Relu,
                                bias=b1t_sb[:, l * KI + mi: l * KI + mi + 1])
                        else:
                            nc.vector.tensor_scalar(
                                out=a_sb[:, mi * TOK:(mi + 1) * TOK], in0=pa[:],
                                scalar1=b1t_sb[:, l * KI + mi: l * KI + mi + 1],
                                scalar2=0.0, op0=ALU.add, op1=ALU.max)

                    for u in range(2):
                        px2 = pproj()
                        for ti in range(2):
                            t = 2 * u + ti
                            xt = x_big[:, (b0 + t) * D:(b0 + t + 1) * D]
                            nc.tensor.matmul(out=px2[:, ti * D:(ti + 1) * D],
                                             lhsT=ident[:], rhs=xt,
                                             start=(ti == 0), stop=False)
                            nc.tensor.matmul(out=px2[:, ti * D:(ti + 1) * D],
                                             lhsT=ones_rbf[:, 0:P],
                                             rhs=b2r_sb[:, l * D:(l + 1) * D],
                                             start=False, stop=False)
                            a3 = a_sb[:].rearrange(
                                "p (ki tok) -> p ki tok", ki=KI)
                            w23 = w2_sb[l][:].rearrange(
                                "p (ki d) -> p ki d", ki=KI)
                            for kp in range(0, KI, 2):
                                nc.tensor.matmul(
                                    out=px2[:, ti * D:(ti + 1) * D],
                                    lhsT=a3[:, kp:kp + 2, t * P:(t + 1) * P],
                                    rhs=w23[:, kp:kp + 2, :], perf_mode=DR,
                                    start=False,
                                    stop=(ti == 1 and kp == KI - 2))
                        xpair = x_big[:, (b0 + 2 * u) * D:(b0 + 2 * u + 2) * D]
                        if u == 0:
                            nc.scalar.copy(out=xpair, in_=px2[:])
                        else:
                            nc.vector.tensor_copy(out=xpair, in_=px2[:])

            # ================= final LN + head ==================================
            xl = cp.tile([BPC, D], BF16)
            for b in range(BPC):
                nc.sync.dma_start(out=xl[b:b + 1, :],
                                  in_=x_big[P - 1:P, b * D:(b + 1) * D])
            stf = cp.tile([BPC, 6], F32)
            nc.vector.bn_stats(out=stf[:], in_=xl[:])
            agf = cp.tile([BPC, 2], F32)
            nc.vector.bn_aggr(out=agf[:], in_=stf[:])
            lnvf = cp.tile([BPC, 1], F32)
            nc.scalar.activation(out=lnvf[:], in_=agf[:, 1:2], func=AF.Ln,
                                 bias=eps_c[0:BPC])
            rsf = cp.tile([BPC, 1], F32)
            nc.scalar.activation(out=rsf[:], in_=lnvf[:], func=AF.Exp, scale=-0.5)
            nc.vector.tensor_tensor(out=rsf[:], in0=rsf[:], in1=onem32[:], op=ALU.mult)
            nmuf = cp.tile([BPC, 1], F32)
            nc.vector.scalar_tensor_tensor(out=nmuf[:], in0=agf[:, 0:1], scalar=-1.0,
                                           in1=rsf[:], op0=ALU.mult, op1=ALU.mult)
            xls = cp.tile([BPC, D], BF16)
            nc.scalar.activation(out=xls[:], in_=xl[:], func=AF.Identity,
                                 bias=nmuf[:], scale=rsf[:])
            xlT = cp.tile([P, KD * BPC], F8)
            mps = pmisc()
            for c in range(KD):
                nc.tensor.transpose(out=mps[:, c * BPC:(c + 1) * BPC],
                                    in_=xls[:, c * P:(c + 1) * P],
                                    identity=ident[0:BPC, 0:BPC])
            nc.scalar.copy(out=xlT[:], in_=mps[:, 0:KD * BPC])

            for ns in range(0, NVC, WSLAB):
                wt = hp.tile([P, KD * WSLAB * NV], F8, tag="wout", name="wout")
                for kc in range(KD):
                    nc.sync.dma_start(
                        out=wt[:, kc * WSLAB * NV:(kc + 1) * WSLAB * NV],
                        in_=wout_d[kc * P:(kc + 1) * P, ns * NV:(ns + WSLAB) * NV])
                if with_bout:
                    bt = hp.tile([1, WSLAB * NV], BF16, tag="boutt", name="boutt")
                    nc.sync.dma_start(out=bt[:], in_=bout_d[:, ns * NV:(ns + WSLAB) * NV])
                xlT3 = xlT[:].rearrange("p (kc b) -> p kc b", kc=KD)
                wt3 = wt[:].rearrange("p (kc w) -> p kc w", kc=KD)
                for si in range(WSLAB):
                    n = ns + si
                    plog = psD.tile([BPC, NV], F32, tag="denoT", name="plog")
                    nc.tensor.matmul(
                        out=plog[:], lhsT=xlT3,
                        rhs=wt3[:, :, si * NV:(si + 1) * NV], perf_mode=DR,
                        start=True, stop=not with_bout)
                    if with_bout:
                        nc.tensor.matmul(out=plog[:], lhsT=onemw_row[:],
                                         rhs=bt[:, si * NV:(si + 1) * NV],
                                         start=False, stop=True)
                    lsb = hp.tile([BPC, NV], F32, tag="lsb", name="lsb")
                    if n % 2 == 0:
                        nc.scalar.copy(out=lsb[:], in_=plog[:])
                    else:
                        nc.vector.tensor_copy(out=lsb[:], in_=plog[:])
                    di = nc.sync.dma_start(out=out_d[:, n * NV:(n + 1) * NV], in_=lsb[:])
                    head_dma_insts.append(di.ins)

            # ================= boost RMW scatter-add ============================
            out_flat = out_d[:].rearrange("a v -> (a v)").rearrange("(n c) -> n c", c=BLK)
            for r in range(BCAP // P):
                bi = gp.tile([P, 1], I32, tag="bidx", name="bidx")
                nc.sync.dma_start(out=bi[:], in_=bidx_d[r * P:(r + 1) * P])
                br = gp.tile([P, BLK], F32, tag="brow", name="brow")
                nc.sync.dma_start(out=br[:], in_=brows_d[r * P:(r + 1) * P, :])
                g = gp.tile([P, BLK], F32, tag="grmw", name="grmw")
                nc.vector.memset(g[:], 0.0)
                gi = nc.gpsimd.indirect_dma_start(
                    out=g[:], out_offset=None, in_=out_flat,
                    in_offset=IndirectOffsetOnAxis(ap=bi[:, :1], axis=0),
                    bounds_check=NBLK - 1, oob_is_err=False)
                for di in head_dma_insts:
                    tile.add_dep_helper(gi.ins, di, reason="boost RMW after head DMA")
                nc.vector.scalar_tensor_tensor(out=g[:], in0=br[:], scalar=sbc[:],
                                               in1=g[:], op0=ALU.mult, op1=ALU.add)
                nc.gpsimd.indirect_dma_start(
                    out=out_flat, out_offset=IndirectOffsetOnAxis(ap=bi[:, :1], axis=0),
                    in_=g[:], in_offset=None,
                    bounds_check=NBLK - 1, oob_is_err=False)

    nc.compile()
    return nc


_CACHE = {}


def _get_nc(with_ln1_bias: bool, with_bout: bool, reps: int = 1):
    key = (bool(with_ln1_bias), bool(with_bout), reps)
    if key not in _CACHE:
        _CACHE[key] = _build(with_ln1_bias, with_bout, reps)
    return _CACHE[key]


def _prep_inputs(inputs):
    """Host-side preprocessing: returns (in_maps, with_ln1_bias, with_bout)."""
    f = lambda a: np.asarray(a, dtype=np.float32)
    bf = lambda a: np.ascontiguousarray(a).astype(ml_dtypes.bfloat16)
    f8np = mybir.dt.np(mybir.dt.float8e4)
    f8 = lambda a: np.clip(np.ascontiguousarray(a), -240.0, 240.0).astype(f8np)

    locations = np.asarray(inputs["locations"]).astype(np.int64)
    users = np.asarray(inputs["users"]).astype(np.int64)
    loc_emb = f(inputs["loc_emb"])
    user_emb = f(inputs["user_emb"])
    Wq, Wk, Wv, Wo = (f(inputs[k]) for k in ("Wq", "Wk", "Wv", "Wo"))
    W1, W2 = f(inputs["W1"]), f(inputs["W2"])
    b1, b2 = f(inputs["b1"]), f(inputs["b2"])
    ln1_g, ln1_b = f(inputs["ln1_g"]), f(inputs["ln1_b"])
    ln2_g, ln2_b = f(inputs["ln2_g"]), f(inputs["ln2_b"])
    lnf_g, lnf_b = f(inputs["lnf_g"]), f(inputs["lnf_b"])
    W_out, b_out = f(inputs["W_out"]), f(inputs["b_out"])
    position_boost = f(inputs["position_boost"])
    return_strength = f(inputs["return_strength"]).reshape(1, 1)
    ensemble_weight = f(inputs["ensemble_weight"]).reshape(1, 1)

    # LN folds (exact linear algebra, done once on host)
    wq_e = ln1_g[:, :, None] * Wq
    wk_e = ln1_g[:, :, None] * Wk
    wv_e = ln1_g[:, :, None] * Wv
    w1_e = ln2_g[:, :, None] * W1
    b1_e = b1 + np.einsum("ld,ldj->lj", ln2_b, W1)
    wout_e = lnf_g[:, None] * W_out
    bout_e = b_out + lnf_b @ W_out
    qrow = np.einsum("ld,ldj->lj", ln1_b, Wq)
    krow = np.einsum("ld,ldj->lj", ln1_b, Wk)
    vrow = np.einsum("ld,ldj->lj", ln1_b, Wv)
    with_ln1_bias = bool(max(np.abs(qrow).max(), np.abs(krow).max(),
                             np.abs(vrow).max()) > 0)
    with_bout = bool(np.abs(bout_e).max() > 0)
    lnb_rows = np.stack([qrow, krow, vrow])[:, :, None, :]  # [3, L, 1, D]

    b1t = b1_e.reshape(L * KI, P).T.copy()  # [P, L*KI]
    posenc = _posenc()

    w = np.float32(1.0 / (1.0 + np.exp(-np.float64(ensemble_weight[0, 0]))))
    onem = np.float32(1.0) - w
    sboost = w * return_strength[0, 0]

    shared = {
        "lemb": bf(loc_emb),
        "uemb": bf(user_emb),
        "posenc": bf(posenc),
        "wq": f8(wq_e), "wk": f8(wk_e), "wv": f8(wv_e), "wo": f8(Wo),
        "w1": f8(w1_e), "w2": f8(W2),
        "b1t": b1t, "b2r": bf(b2.reshape(1, L * D)),
        "lnbrows": bf(lnb_rows),
        "wout": f8(wout_e), "bout": bf(bout_e[None, :]),
        "onem32": np.full((BPC, 1), onem, np.float32),
        "sbcv": np.full((P, 1), sboost, np.float32),
        "onemw_row": np.full((1, BPC), onem, ml_dtypes.bfloat16),
    }

    in_maps = []
    for c in range(NCORES):
        lc = locations[c * BPC:(c + 1) * BPC]            # [BPC, S]
        uc = users[c * BPC:(c + 1) * BPC]
        rows = {}
        for b in range(BPC):
            for j in range(NB):
                col = int(lc[b, S - 1 - j])
                flat = b * V + col
                blk, off = flat // BLK, flat % BLK
                if blk not in rows:
                    rows[blk] = np.zeros(BLK, np.float32)
                rows[blk][off] += position_boost[j]
        bidx = np.full((BCAP, 1), 1 << 20, np.int32)
        brows = np.zeros((BCAP, BLK), np.float32)
        for i, (blk, row) in enumerate(sorted(rows.items())):
            bidx[i, 0] = blk
            brows[i] = row
        m = dict(shared)
        m["locst"] = np.ascontiguousarray(lc.astype(np.int32).T)       # [S, BPC]
        m["userst"] = np.ascontiguousarray(
            np.repeat(uc.astype(np.int32)[None, :], P, axis=0))        # [P, BPC]
        m["bidx"] = bidx
        m["brows"] = brows
        in_maps.append(m)
    return in_maps, with_ln1_bias, with_bout


def kernel(**inputs) -> np.ndarray:
    in_maps, with_ln1_bias, with_bout = _prep_inputs(inputs)
    nc = _get_nc(with_ln1_bias, with_bout)
    res = bass_utils.run_bass_kernel_spmd(nc, in_maps, core_ids=list(range(NCORES)))
    return np.concatenate([r["out"] for r in res.results], axis=0)


# revision 26
# speedup vs baseline: 6.6831x; 1.0002x over previous
"""Trainium2 Bass kernel for nn_EnhancedBaselineWithReturnBoost.

4-layer transformer encoder (D=256, H=8, DI=1024) over [B=256, S=128] location
sequences, final-token head into V=50000 logits, plus a scatter-add "return
boost" on recent locations, ensembled with sigmoid(ensemble_weight).

Sharding: pure data-parallel over batch across 8 NeuronCores (32 batch items
per core).  One batch item = one 128-token tile (S=128); tiles are processed
in QUADS (4 batch items, 512-wide moving dimension) to amortize instruction
overheads.  The residual stream x is bf16 in SBUF; matmuls run bf16 with fp32
PSUM accumulation.

Key structural choices:
- LN apply + transpose fused into PE: h^T = x^T @ diag(rs) + 1 (x) nmu_row, so
  the normalized, transposed activations come straight out of the tensor
  engine (no separate ACT apply, no separate transpose pass).
- LN stats (sum, sum of squares) computed on the otherwise-idle GPSIMD engine
  via scalar_tensor_tensor accum_out; small [128,4] fixups on DVE/ACT.
- Attention scores for 4 heads share one [128,512] PSUM bank; a single ACT exp
  evacuates each group.  Softmax denominators via ones-matmul (PE) into the
  same layout as O^T; one reciprocal + one multiply per tile.
- Residual adds are folded into the projection matmuls by injecting x into
  PSUM with an identity matmul; the PSUM->SBUF evacuation writes the new x.
- ACT table thrash eliminated by pinning Exp/Ln to the combined
  natural_log_exp_and_others set (see _pinned_act_tables below).
- Head ([D,V] streamed bf16) and the boost RMW scatter as in v1.
"""
import numpy as np
import ml_dtypes
from contextlib import ExitStack

import concourse.bass as bass
import concourse.mybir as mybir
import concourse.tile as tile
from concourse import bacc, bass_utils
from concourse.bass import IndirectOffsetOnAxis
from concourse.masks import make_identity

# ---------------------------------------------------------------------------
# Pin Exp/Ln to the one activation-table set that contains both
# ("natural_log_exp_and_others").  The stock table-load pass picks the FIRST
# set containing each function ("exp_and_others" for Exp, "natural_log" for
# Ln), which forces an ACT table reload (~1.3us) on every Ln->Exp transition
# — 514 reloads/core in the v1 kernel.  Claiming Exp/Ln only in the combined
# set (a valid id in act_info.json that really contains exp+ln+identity+copy)
# makes the pass emit a single load.  Compile-time metadata only; the emitted
# act_func_set_id remains correct for walrus/runtime.
import concourse.bacc as _bacc_mod

_AF = mybir.ActivationFunctionType
_orig_get_act_tables = _bacc_mod.get_activation_tables


def _pinned_act_tables(arch):
    out = {}
    for name, funcs in _orig_get_act_tables(arch).items():
        if name != "natural_log_exp_and_others":
            funcs = set(funcs) - {_AF.Exp, _AF.Ln}
        out[name] = set(funcs)
    return out


_bacc_mod.get_activation_tables = _pinned_act_tables

F32 = mybir.dt.float32
BF16 = mybir.dt.bfloat16
F8 = mybir.dt.float8e4
DR = mybir.MatmulPerfMode.DoubleRow
I32 = mybir.dt.int32
AF = mybir.ActivationFunctionType
ALU = mybir.AluOpType

# problem dims (hardcoded per spec)
V, U, D, DI, L, H, B, S = 50000, 1024, 256, 1024, 4, 8, 256, 128
DK = D // H            # 32
NB = 5                 # boost positions
NCORES = 8
BPC = B // NCORES      # 32 batch items per core
P = 128
KD = D // P            # 2 contraction chunks for D
KI = DI // P           # 8 chunks for DI
QUAD = 4               # batch tiles per processing group
TOK = QUAD * P         # 512-wide moving dimension
NQ = BPC // QUAD       # 8 quads per core
NV = 500               # head logits chunk width
NVC = V // NV          # 100 chunks
WSLAB = 4              # head chunks per streamed weight tile
BLK = 256              # boost scatter block (f32 elems) = 1KB
NBLK = BPC * V // BLK  # 6250 blocks per core
BCAP = 256             # max boost rows per core (>= BPC*NB=160), 2 rounds of 128
ATT_SCALE = 1.0 / np.sqrt(DK)
EPS = 1e-5


def _posenc():
    pos = np.arange(S)[:, None]
    i = np.arange(0, D, 2)[None, :]
    ang = pos / np.power(10000.0, i / D)
    pe = np.zeros((S, D), dtype=np.float32)
    pe[:, 0::2] = np.sin(ang)
    pe[:, 1::2] = np.cos(ang)
    return pe


def _build(with_ln1_bias: bool, with_bout: bool, reps: int = 1):
    """Build + compile the per-core Bass program (SPMD: same NEFF, per-core data)."""
    nc = bacc.Bacc("TRN2", target_bir_lowering=False, debug=False, num_devices=NCORES)

    # ---- DRAM I/O -----------------------------------------------------------
    locst_d = nc.dram_tensor("locst", [P, BPC], I32, kind="ExternalInput")
    userst_d = nc.dram_tensor("userst", [P, BPC], I32, kind="ExternalInput")
    lemb_d = nc.dram_tensor("lemb", [V, D], BF16, kind="ExternalInput")
    uemb_d = nc.dram_tensor("uemb", [U, D], BF16, kind="ExternalInput")
    pos_d = nc.dram_tensor("posenc", [S, D], BF16, kind="ExternalInput")
    wq_d = nc.dram_tensor("wq", [L, D, D], F8, kind="ExternalInput")
    wk_d = nc.dram_tensor("wk", [L, D, D], F8, kind="ExternalInput")
    wv_d = nc.dram_tensor("wv", [L, D, D], F8, kind="ExternalInput")
    wo_d = nc.dram_tensor("wo", [L, D, D], F8, kind="ExternalInput")
    w1_d = nc.dram_tensor("w1", [L, D, DI], F8, kind="ExternalInput")
    w2_d = nc.dram_tensor("w2", [L, DI, D], F8, kind="ExternalInput")
    b1t_d = nc.dram_tensor("b1t", [P, L * KI], F32, kind="ExternalInput")
    b2r_d = nc.dram_tensor("b2r", [1, L * D], BF16, kind="ExternalInput")
    lnb_d = nc.dram_tensor("lnbrows", [3, L, 1, D], BF16, kind="ExternalInput")
    wout_d = nc.dram_tensor("wout", [D, V], F8, kind="ExternalInput")
    bout_d = nc.dram_tensor("bout", [1, V], BF16, kind="ExternalInput")
    onem_d = nc.dram_tensor("onem32", [BPC, 1], F32, kind="ExternalInput")
    sbc_d = nc.dram_tensor("sbcv", [P, 1], F32, kind="ExternalInput")
    onemr_d = nc.dram_tensor("onemw_row", [1, BPC], BF16, kind="ExternalInput")
    bidx_d = nc.dram_tensor("bidx", [BCAP, 1], I32, kind="ExternalInput")
    brows_d = nc.dram_tensor("brows", [BCAP, BLK], F32, kind="ExternalInput")
    out_d = nc.dram_tensor("out", [BPC, V], F32, kind="ExternalOutput")

    head_dma_insts = []

    with tile.TileContext(nc) as tc, ExitStack() as ctx:
        cp = ctx.enter_context(tc.tile_pool(name="const", bufs=1))
        wp = ctx.enter_context(tc.tile_pool(name="wts", bufs=1))
        sp = ctx.enter_context(tc.tile_pool(name="work", bufs=3))
        ap_ = ctx.enter_context(tc.tile_pool(name="attw", bufs=3))
        gp = ctx.enter_context(tc.tile_pool(name="gath", bufs=4))
        hp = ctx.enter_context(tc.tile_pool(name="head", bufs=2))
        # PSUM: 8 banks = psP 3x[128,512] + psS 2 (scores) + psD 3 (den/oT 2
        # + misc 1).  PSUM-bank rule (HW-probed): matmuls with different
        # tile_position row groups run concurrently on different sub-arrays
        # and must NOT share a destination bank; same row group serializes
        # and may share.
        psP = ctx.enter_context(tc.tile_pool(name="psP", bufs=3, space="PSUM"))
        psS = ctx.enter_context(tc.tile_pool(name="psS", bufs=2, space="PSUM"))
        psD = ctx.enter_context(tc.tile_pool(name="psD", bufs=2, space="PSUM"))

        def pproj():
            return psP.tile([P, TOK], F32, tag="proj", name="pproj")

        def pscore():
            return psS.tile([P, TOK], F32, tag="score", name="pscore")

        def pmisc(dtype=BF16):
            return psD.tile([P, TOK], dtype, tag="misc", name="pmisc", bufs=1)

        # ---- constants ------------------------------------------------------
        ident = cp.tile([P, P], BF16)
        make_identity(nc, ident[:])
        ones_rbf = cp.tile([1, TOK], BF16)
        nc.vector.memset(ones_rbf[:], 1.0)
        ones_m32 = cp.tile([P, DK], BF16)
        nc.vector.memset(ones_m32[:], 1.0)
        eps_c = cp.tile([P, 1], F32)
        nc.vector.memset(eps_c[:], EPS)

        pos_sb = cp.tile([P, D], BF16)
        nc.sync.dma_start(out=pos_sb[:], in_=pos_d[:])
        b1t_sb = cp.tile([P, L * KI], F32)
        nc.sync.dma_start(out=b1t_sb[:], in_=b1t_d[:])
        b2r_sb = cp.tile([1, L * D], BF16)
        nc.sync.dma_start(out=b2r_sb[:], in_=b2r_d[:])
        locst_sb = cp.tile([P, BPC], I32)
        nc.sync.dma_start(out=locst_sb[:], in_=locst_d[:])
        userst_sb = cp.tile([P, BPC], I32)
        nc.sync.dma_start(out=userst_sb[:], in_=userst_d[:])
        lnb_sb = None
        if with_ln1_bias:
            lnb_sb = cp.tile([1, 3 * L * D], BF16)
            for t in range(3):
                for l in range(L):
                    nc.sync.dma_start(
                        out=lnb_sb[:, (t * L + l) * D:(t * L + l + 1) * D],
                        in_=lnb_d[t, l],
                    )

        # ---- weights resident in SBUF --------------------------------------
        wq_sb, wk_sb, wv_sb, wo_sb, w1_sb, w2_sb = [], [], [], [], [], []
        for l in range(L):
            for (nm, lst, dram, width) in (
                ("wq", wq_sb, wq_d, D), ("wk", wk_sb, wk_d, D),
                ("wv", wv_sb, wv_d, D), ("wo", wo_sb, wo_d, D),
            ):
                t = wp.tile([P, KD * width], F8, tag=f"{nm}_{l}", name=f"{nm}_{l}")
                for kc in range(KD):
                    nc.sync.dma_start(
                        out=t[:, kc * width:(kc + 1) * width],
                        in_=dram[l, kc * P:(kc + 1) * P, :],
                    )
                lst.append(t)
            t = wp.tile([P, KD * DI], F8, tag=f"w1_{l}", name=f"w1_{l}")
            for kc in range(KD):
                nc.sync.dma_start(out=t[:, kc * DI:(kc + 1) * DI],
                                  in_=w1_d[l, kc * P:(kc + 1) * P, :])
            w1_sb.append(t)
            t = wp.tile([P, KI * D], F8, tag=f"w2_{l}", name=f"w2_{l}")
            for ki in range(KI):
                nc.sync.dma_start(out=t[:, ki * D:(ki + 1) * D],
                                  in_=w2_d[l, ki * P:(ki + 1) * P, :])
            w2_sb.append(t)

        # ---- ensemble scalars (computed host-side, DMA'd in) ----------------
        onem32 = cp.tile([BPC, 1], F32)
        nc.sync.dma_start(out=onem32[:], in_=onem_d[:])
        sbc = cp.tile([P, 1], F32)
        nc.sync.dma_start(out=sbc[:], in_=sbc_d[:])
        if with_bout:
            onemw_row = cp.tile([1, BPC], BF16)
            nc.sync.dma_start(out=onemw_row[:], in_=onemr_d[:])

        # ---- residual stream ------------------------------------------------
        x_big = cp.tile([P, BPC * D], BF16)   # x for all 32 batch tiles

        def ln_to_hT(b0, which):
            """LN of x tiles b0..b0+3 fused into transposed output:
            hT[:, kc*TOK + t*P + tok] = ((x[tok, kc*128+f] - mu) * rs).

            Stats on GPSIMD (accum_out), fixups on DVE, Ln/Exp on ACT,
            h^T = x^T @ diag(rs) + ones^T (x) nmu_row on PE."""
            agq = sp.tile([P, 2 * QUAD], F32, tag=f"lnag{which}", name="agq")
            for t in range(QUAD):
                xt = x_big[:, (b0 + t) * D:(b0 + t + 1) * D]
                st = sp.tile([P, 6], F32, tag="lnst", name="st", bufs=4)
                nc.vector.bn_stats(out=st[:], in_=xt)
                nc.vector.bn_aggr(out=agq[:, 2 * t:2 * t + 2], in_=st[:])
            lnv = sp.tile([P, QUAD], F32, tag=f"lnlnv{which}", name="lnv")
            nc.scalar.activation(out=lnv[:], in_=agq[:, 1:2 * QUAD:2], func=AF.Ln,
                                 bias=eps_c[:])
            rs_q = sp.tile([P, QUAD], F32, tag=f"lnrs{which}", name="rs_q")
            nc.scalar.activation(out=rs_q[:], in_=lnv[:], func=AF.Exp, scale=-0.5)
            nmu_q = sp.tile([P, QUAD], BF16, tag=f"lnnmu{which}", name="nmu_q")
            nc.vector.scalar_tensor_tensor(out=nmu_q[:], in0=agq[:, 0:2 * QUAD:2],
                                           scalar=-1.0, in1=rs_q[:],
                                           op0=ALU.mult, op1=ALU.mult)
            # nmu as a row [1, TOK]: per-tile PE transpose of [128,1] -> [1,128]
            nmt_ps = pmisc()
            for t in range(QUAD):
                nc.tensor.transpose(out=nmt_ps[0:1, t * P:(t + 1) * P],
                                    in_=nmu_q[:, t:t + 1], identity=ident[:])
            nmursT = sp.tile([1, TOK], BF16, tag=f"lnrow{which}", name="nmursT")
            nc.vector.tensor_copy(out=nmursT[:], in_=nmt_ps[0:1, :])

            diags = []
            for t in range(QUAD):
                dg = sp.tile([P, P], BF16, tag="lndiag", name="dg", bufs=8)
                nc.vector.tensor_scalar(out=dg[:], in0=ident[:],
                                        scalar1=rs_q[:, t:t + 1], scalar2=None,
                                        op0=ALU.mult)
                diags.append(dg)

            hT = sp.tile([P, KD * TOK], F8, tag=f"hT{which}", name="hT")
            hTv = hT[:].rearrange("p (kc t tok) -> p kc t tok", kc=KD, t=QUAD, tok=P)
            for u in range(2):
                hps = pproj()
                for ti in range(2):
                    t = 2 * u + ti
                    xt = x_big[:, (b0 + t) * D:(b0 + t + 1) * D]
                    for kc in range(KD):
                        off = ti * 2 * P + kc * P
                        nc.tensor.matmul(
                            out=hps[:, off:off + P],
                            lhsT=xt[:, kc * P:(kc + 1) * P], rhs=diags[t][:],
                            start=(ti == 0 and kc == 0), stop=False)
                        nc.tensor.matmul(
                            out=hps[:, off:off + P],
                            lhsT=ones_rbf[:, 0:P], rhs=nmursT[:, t * P:(t + 1) * P],
                            start=False, stop=(ti == 1 and kc == KD - 1))
                # evac: bank layout (ti, kc, tok) -> hT layout (kc, t, tok)
                dst = hTv[:, :, 2 * u:2 * u + 2, :].rearrange(
                    "p kc t tok -> p t kc tok")
                src = hps[:].rearrange("p (t kc tok) -> p t kc tok",
                                       t=2, kc=KD, tok=P)
                if u == 0:
                    nc.scalar.copy(out=dst, in_=src)
                else:
                    nc.vector.tensor_copy(out=dst, in_=src)
            return hT

        for _rep in range(reps):
            head_dma_insts = []
            # ================= per-quad pipeline ================================
            for q in range(NQ):
                b0 = QUAD * q
                # ---------- embeddings ----------
                for t in range(QUAD):
                    b = b0 + t
                    xt = x_big[:, b * D:(b + 1) * D]
                    xg = gp.tile([P, D], BF16, tag="xg", name="xg")
                    nc.gpsimd.indirect_dma_start(
                        out=xg[:], out_offset=None, in_=lemb_d[:],
                        in_offset=IndirectOffsetOnAxis(ap=locst_sb[:, b:b + 1], axis=0))
                    ub = gp.tile([P, D], BF16, tag="ub", name="ub")
                    nc.gpsimd.indirect_dma_start(
                        out=ub[:], out_offset=None, in_=uemb_d[:],
                        in_offset=IndirectOffsetOnAxis(ap=userst_sb[:, b:b + 1], axis=0))
                    nc.vector.tensor_add(out=xt, in0=xg[:], in1=pos_sb[:])
                    nc.vector.tensor_add(out=xt, in0=xt, in1=ub[:])

                for l in range(L):
                    # ---------- LN1 -> h1T ----------
                    h1T = ln_to_hT(b0, 1)

                    # ---------- Q,K projections (feature-major, N=TOK) -------
                    qT = sp.tile([P, KD * TOK], BF16, tag="qT", name="qT")
                    kT = sp.tile([P, KD * TOK], BF16, tag="kT", name="kT")
                    h1T3 = h1T[:].rearrange("p (kc tok) -> p kc tok", kc=KD)
                    for pi, (dst, wsb) in enumerate(((qT, wq_sb[l]), (kT, wk_sb[l]))):
                        w3 = wsb[:].rearrange("p (kc d) -> p kc d", kc=KD)
                        for m in range(KD):
                            pq = pproj()
                            nc.tensor.matmul(
                                out=pq[:], lhsT=w3[:, :, m * P:(m + 1) * P],
                                rhs=h1T3, perf_mode=DR,
                                start=True, stop=(lnb_sb is None))
                            if lnb_sb is not None:
                                nc.tensor.matmul(
                                    out=pq[:],
                                    lhsT=lnb_sb[:, (pi * L + l) * D + m * P:
                                                (pi * L + l) * D + (m + 1) * P],
                                    rhs=ones_rbf[:], start=False, stop=True)
                            if (m + pi) % 2 == 0:
                                nc.scalar.copy(out=dst[:, m * TOK:(m + 1) * TOK],
                                               in_=pq[:])
                            else:
                                nc.vector.tensor_copy(
                                    out=dst[:, m * TOK:(m + 1) * TOK], in_=pq[:])

                    # ---------- V (token-major per tile) ----------
                    v_sb = sp.tile([P, QUAD * D], BF16, tag="vsb", name="v_sb")
                    wv3 = wv_sb[l][:].rearrange("p (kc d) -> p kc d", kc=KD)
                    for u in range(2):
                        pv = pproj()
                        for ti in range(2):
                            t = 2 * u + ti
                            nc.tensor.matmul(
                                out=pv[:, ti * D:(ti + 1) * D],
                                lhsT=h1T3[:, :, t * P:(t + 1) * P],
                                rhs=wv3, perf_mode=DR,
                                start=(ti == 0),
                                stop=(ti == 1 and lnb_sb is None))
                            if lnb_sb is not None:
                                nc.tensor.matmul(
                                    out=pv[:, ti * D:(ti + 1) * D],
                                    lhsT=ones_rbf[:, 0:P],
                                    rhs=lnb_sb[:, (2 * L + l) * D:(2 * L + l + 1) * D],
                                    start=False, stop=(ti == 1))
                        if u == 0:
                            nc.scalar.copy(out=v_sb[:, 0:2 * D], in_=pv[:])
                        else:
                            nc.vector.tensor_copy(out=v_sb[:, 2 * D:4 * D], in_=pv[:])

                    # ---------- attention (per tile-pair) ----------
                    # Scores grouped by head row-group h4: the 4 matmuls for
                    # (tp, mq) share tile_position row group (h4*DK, 0), so
                    # they may legally share one PSUM bank (they serialize).
                    # One exp evacuates each h4 bank.
                    oTn_l = []
                    for u in range(2):
                        att = ap_.tile([P, 4 * TOK], BF16, tag="att", name="att",
                                       bufs=2)
                        for h4 in range(4):
                            po = h4 * DK
                            sc = pscore()
                            for tp in range(2):
                                for mq in range(2):
                                    cs = mq * TOK + (2 * u + tp) * P
                                    nc.tensor.matmul(
                                        out=sc[:, (tp * 2 + mq) * P:
                                               (tp * 2 + mq + 1) * P],
                                        lhsT=kT[po:po + DK, cs:cs + P],
                                        rhs=qT[po:po + DK, cs:cs + P],
                                        start=True, stop=True,
                                        tile_position=(po, 0))
                            nc.scalar.activation(
                                out=att[:, h4 * TOK:(h4 + 1) * TOK], in_=sc[:],
                                func=AF.Exp, scale=ATT_SCALE)
                        for tp in range(2):
                            t = 2 * u + tp
                            # den and oT in separate banks so the reciprocal
                            # (den read) can't collide with oT matmul writes.
                            dn = psD.tile([P, D], F32, tag="denoT", name="dn")
                            ot = psD.tile([P, D], F32, tag="denoT", name="ot")
                            for h in range(H):
                                mq, h4 = h // 4, h % 4
                                po = h4 * DK
                                asl = att[:, h4 * TOK + (tp * 2 + mq) * P:
                                          h4 * TOK + (tp * 2 + mq + 1) * P]
                                nc.tensor.matmul(
                                    out=dn[po:po + DK, mq * P:(mq + 1) * P],
                                    lhsT=ones_m32[:], rhs=asl,
                                    start=True, stop=True,
                                    tile_position=(0, po))
                                nc.tensor.matmul(
                                    out=ot[po:po + DK, mq * P:(mq + 1) * P],
                                    lhsT=v_sb[:, t * D + h * DK:
                                              t * D + (h + 1) * DK],
                                    rhs=asl, start=True, stop=True,
                                    tile_position=(0, po))
                            rf = sp.tile([P, D], F32, tag="rf", name="rf", bufs=4)
                            nc.vector.reciprocal(out=rf[:], in_=dn[:])
                            oTn = sp.tile([P, D], F8, tag="oTn", name="oTn",
                                          bufs=4)
                            nc.vector.tensor_tensor(out=oTn[:], in0=ot[:],
                                                    in1=rf[:], op=ALU.mult)
                            oTn_l.append(oTn)

                    # ---------- O projection + residual (x injected) ----------
                    for u in range(2):
                        pox = pproj()
                        for ti in range(2):
                            t = 2 * u + ti
                            xt = x_big[:, (b0 + t) * D:(b0 + t + 1) * D]
                            nc.tensor.matmul(out=pox[:, ti * D:(ti + 1) * D],
                                             lhsT=ident[:], rhs=xt,
                                             start=(ti == 0), stop=False)
                            oTn3 = oTn_l[t][:].rearrange(
                                "p (kc q) -> p kc q", kc=KD)
                            wo3 = wo_sb[l][:].rearrange(
                                "p (kc d) -> p kc d", kc=KD)
                            nc.tensor.matmul(
                                out=pox[:, ti * D:(ti + 1) * D],
                                lhsT=oTn3, rhs=wo3, perf_mode=DR,
                                start=False, stop=(ti == 1))
                        xpair = x_big[:, (b0 + 2 * u) * D:(b0 + 2 * u + 2) * D]
                        if u == 0:
                            nc.scalar.copy(out=xpair, in_=pox[:])
                        else:
                            nc.vector.tensor_copy(out=xpair, in_=pox[:])

                    # ---------- LN2 -> h2T, FFN ----------
                    h2T = ln_to_hT(b0, 2)

                    a_sb = sp.tile([P, KI * TOK], F8, tag="asb", name="a_sb")
                    h2T3 = h2T[:].rearrange("p (kc tok) -> p kc tok", kc=KD)
                    w13 = w1_sb[l][:].rearrange("p (kc d) -> p kc d", kc=KD)
                    for mi in range(KI):
                        pa = pproj()
                        nc.tensor.matmul(
                            out=pa[:], lhsT=w13[:, :, mi * P:(mi + 1) * P],
                            rhs=h2T3, perf_mode=DR, start=True, stop=True)
                        if mi % 2 == 0:
                            nc.scalar.activation(
                                out=a_sb[:, mi * TOK:(mi + 1) * TOK], in_=pa[:],
                                func=AF.Relu,
                                bias=b1t_sb[:, l * KI + mi: l * KI + mi + 1])
                        else:
                            nc.vector.tensor_scalar(
                                out=a_sb[:, mi * TOK:(mi + 1) * TOK], in0=pa[:],
                                scalar1=b1t_sb[:, l * KI + mi: l * KI + mi + 1],
                                scalar2=0.0, op0=ALU.add, op1=ALU.max)

                    for u in range(2):
                        px2 = pproj()
                        for ti in range(2):
                            t = 2 * u + ti
                            xt = x_big[:, (b0 + t) * D:(b0 + t + 1) * D]
                            nc.tensor.matmul(out=px2[:, ti * D:(ti + 1) * D],
                                             lhsT=ident[:], rhs=xt,
                                             start=(ti == 0), stop=False)
                            nc.tensor.matmul(out=px2[:, ti * D:(ti + 1) * D],
                                             lhsT=ones_rbf[:, 0:P],
                                             rhs=b2r_sb[:, l * D:(l + 1) * D],
                                             start=False, stop=False)
                            a3 = a_sb[:].rearrange(
                                "p (ki tok) -> p ki tok", ki=KI)
                            w23 = w2_sb[l][:].rearrange(
                                "p (ki d) -> p ki d", ki=KI)
                            for kp in range(0, KI, 2):
                                nc.tensor.matmul(
                                    out=px2[:, ti * D:(ti + 1) * D],
                                    lhsT=a3[:, kp:kp + 2, t * P:(t + 1) * P],
                                    rhs=w23[:, kp:kp + 2, :], perf_mode=DR,
                                    start=False,
                                    stop=(ti == 1 and kp == KI - 2))
                        xpair = x_big[:, (b0 + 2 * u) * D:(b0 + 2 * u + 2) * D]
                        if u == 0:
                            nc.scalar.copy(out=xpair, in_=px2[:])
                        else:
                            nc.vector.tensor_copy(out=xpair, in_=px2[:])

            # ================= final LN + head ==================================
            xl = cp.tile([BPC, D], BF16)
            for b in range(BPC):
                nc.sync.dma_start(out=xl[b:b + 1, :],
                                  in_=x_big[P - 1:P, b * D:(b + 1) * D])
            stf = cp.tile([BPC, 6], F32)
            nc.vector.bn_stats(out=stf[:], in_=xl[:])
            agf = cp.tile([BPC, 2], F32)
            nc.vector.bn_aggr(out=agf[:], in_=stf[:])
            lnvf = cp.tile([BPC, 1], F32)
            nc.scalar.activation(out=lnvf[:], in_=agf[:, 1:2], func=AF.Ln,
                                 bias=eps_c[0:BPC])
            rsf = cp.tile([BPC, 1], F32)
            nc.scalar.activation(out=rsf[:], in_=lnvf[:], func=AF.Exp, scale=-0.5)
            nc.vector.tensor_tensor(out=rsf[:], in0=rsf[:], in1=onem32[:], op=ALU.mult)
            nmuf = cp.tile([BPC, 1], F32)
            nc.vector.scalar_tensor_tensor(out=nmuf[:], in0=agf[:, 0:1], scalar=-1.0,
                                           in1=rsf[:], op0=ALU.mult, op1=ALU.mult)
            xls = cp.tile([BPC, D], BF16)
            nc.scalar.activation(out=xls[:], in_=xl[:], func=AF.Identity,
                                 bias=nmuf[:], scale=rsf[:])
            xlT = cp.tile([P, KD * BPC], F8)
            mps = pmisc()
            for c in range(KD):
                nc.tensor.transpose(out=mps[:, c * BPC:(c + 1) * BPC],
                                    in_=xls[:, c * P:(c + 1) * P],
                                    identity=ident[0:BPC, 0:BPC])
            nc.scalar.copy(out=xlT[:], in_=mps[:, 0:KD * BPC])

            for ns in range(0, NVC, WSLAB):
                wt = hp.tile([P, KD * WSLAB * NV], F8, tag="wout", name="wout")
                for kc in range(KD):
                    nc.sync.dma_start(
                        out=wt[:, kc * WSLAB * NV:(kc + 1) * WSLAB * NV],
                        in_=wout_d[kc * P:(kc + 1) * P, ns * NV:(ns + WSLAB) * NV])
                if with_bout:
                    bt = hp.tile([1, WSLAB * NV], BF16, tag="boutt", name="boutt")
                    nc.sync.dma_start(out=bt[:], in_=bout_d[:, ns * NV:(ns + WSLAB) * NV])
                xlT3 = xlT[:].rearrange("p (kc b) -> p kc b", kc=KD)
                wt3 = wt[:].rearrange("p (kc w) -> p kc w", kc=KD)
                for si in range(WSLAB):
                    n = ns + si
                    plog = psD.tile([BPC, NV], F32, tag="denoT", name="plog")
                    nc.tensor.matmul(
                        out=plog[:], lhsT=xlT3,
                        rhs=wt3[:, :, si * NV:(si + 1) * NV], perf_mode=DR,
                        start=True, stop=not with_bout)
                    if with_bout:
                        nc.tensor.matmul(out=plog[:], lhsT=onemw_row[:],
                                         rhs=bt[:, si * NV:(si + 1) * NV],
                                         start=False, stop=True)
                    lsb = hp.tile([BPC, NV], F32, tag="lsb", name="lsb")
                    if n % 2 == 0:
                        nc.scalar.copy(out=lsb[:], in_=plog[:])
                    else:
                        nc.vector.tensor_copy(out=lsb[:], in_=plog[:])
                    di = nc.sync.dma_start(out=out_d[:, n * NV:(n + 1) * NV], in_=lsb[:])
                    head_dma_insts.append(di.ins)

            # ================= boost RMW scatter-add ============================
            out_flat = out_d[:].rearrange("a v -> (a v)").rearrange("(n c) -> n c", c=BLK)
            for r in range(BCAP // P):
                bi = gp.tile([P, 1], I32, tag="bidx", name="bidx")
                nc.sync.dma_start(out=bi[:], in_=bidx_d[r * P:(r + 1) * P])
                br = gp.tile([P, BLK], F32, tag="brow", name="brow")
                nc.sync.dma_start(out=br[:], in_=brows_d[r * P:(r + 1) * P, :])
                g = gp.tile([P, BLK], F32, tag="grmw", name="grmw")
                nc.vector.memset(g[:], 0.0)
                gi = nc.gpsimd.indirect_dma_start(
                    out=g[:], out_offset=None, in_=out_flat,
                    in_offset=IndirectOffsetOnAxis(ap=bi[:, :1], axis=0),
                    bounds_check=NBLK - 1, oob_is_err=False)
                for di in head_dma_insts:
                    tile.add_dep_helper(gi.ins, di, reason="boost RMW after head DMA")
                nc.vector.scalar_tensor_tensor(out=g[:], in0=br[:], scalar=sbc[:],
                                               in1=g[:], op0=ALU.mult, op1=ALU.add)
                nc.gpsimd.indirect_dma_start(
                    out=out_flat, out_offset=IndirectOffsetOnAxis(ap=bi[:, :1], axis=0),
                    in_=g[:], in_offset=None,
                    bounds_check=NBLK - 1, oob_is_err=False)

    nc.compile()
    return nc


_CACHE = {}


def _get_nc(with_ln1_bias: bool, with_bout: bool, reps: int = 1):
    key = (bool(with_ln1_bias), bool(with_bout), reps)
    if key not in _CACHE:
        _CACHE[key] = _build(with_ln1_bias, with_bout, reps)
    return _CACHE[key]


def _prep_inputs(inputs):
    """Host-side preprocessing: returns (in_maps, with_ln1_bias, with_bout)."""
    f = lambda a: np.asarray(a, dtype=np.float32)
    bf = lambda a: np.ascontiguousarray(a).astype(ml_dtypes.bfloat16)
    f8np = mybir.dt.np(mybir.dt.float8e4)
    f8 = lambda a: np.clip(np.ascontiguousarray(a), -240.0, 240.0).astype(f8np)

    locations = np.asarray(inputs["locations"]).astype(np.int64)
    users = np.asarray(inputs["users"]).astype(np.int64)
    loc_emb = f(inputs["loc_emb"])
    user_emb = f(inputs["user_emb"])
    Wq, Wk, Wv, Wo = (f(inputs[k]) for k in ("Wq", "Wk", "Wv", "Wo"))
    W1, W2 = f(inputs["W1"]), f(inputs["W2"])
    b1, b2 = f(inputs["b1"]), f(inputs["b2"])
    ln1_g, ln1_b = f(inputs["ln1_g"]), f(inputs["ln1_b"])
    ln2_g, ln2_b = f(inputs["ln2_g"]), f(inputs["ln2_b"])
    lnf_g, lnf_b = f(inputs["lnf_g"]), f(inputs["lnf_b"])
    W_out, b_out = f(inputs["W_out"]), f(inputs["b_out"])
    position_boost = f(inputs["position_boost"])
    return_strength = f(inputs["return_strength"]).reshape(1, 1)
    ensemble_weight = f(inputs["ensemble_weight"]).reshape(1, 1)

    # LN folds (exact linear algebra, done once on host)
    wq_e = ln1_g[:, :, None] * Wq
    wk_e = ln1_g[:, :, None] * Wk
    wv_e = ln1_g[:, :, None] * Wv
    w1_e = ln2_g[:, :, None] * W1
    b1_e = b1 + np.einsum("ld,ldj->lj", ln2_b, W1)
    wout_e = lnf_g[:, None] * W_out
    bout_e = b_out + lnf_b @ W_out
    qrow = np.einsum("ld,ldj->lj", ln1_b, Wq)
    krow = np.einsum("ld,ldj->lj", ln1_b, Wk)
    vrow = np.einsum("ld,ldj->lj", ln1_b, Wv)
    with_ln1_bias = bool(max(np.abs(qrow).max(), np.abs(krow).max(),
                             np.abs(vrow).max()) > 0)
    with_bout = bool(np.abs(bout_e).max() > 0)
    lnb_rows = np.stack([qrow, krow, vrow])[:, :, None, :]  # [3, L, 1, D]

    b1t = b1_e.reshape(L * KI, P).T.copy()  # [P, L*KI]
    posenc = _posenc()

    w = np.float32(1.0 / (1.0 + np.exp(-np.float64(ensemble_weight[0, 0]))))
    onem = np.float32(1.0) - w
    sboost = w * return_strength[0, 0]

    shared = {
        "lemb": bf(loc_emb),
        "uemb": bf(user_emb),
        "posenc": bf(posenc),
        "wq": f8(wq_e), "wk": f8(wk_e), "wv": f8(wv_e), "wo": f8(Wo),
        "w1": f8(w1_e), "w2": f8(W2),
        "b1t": b1t, "b2r": bf(b2.reshape(1, L * D)),
        "lnbrows": bf(lnb_rows),
        "wout": f8(wout_e), "bout": bf(bout_e[None, :]),
        "onem32": np.full((BPC, 1), onem, np.float32),
        "sbcv": np.full((P, 1), sboost, np.float32),
        "onemw_row": np.full((1, BPC), onem, ml_dtypes.bfloat16),
    }

    in_maps = []
    for c in range(NCORES):
        lc = locations[c * BPC:(c + 1) * BPC]            # [BPC, S]
        uc = users[c * BPC:(c + 1) * BPC]
        rows = {}
        for b in range(BPC):
            for j in range(NB):
                col = int(lc[b, S - 1 - j])
                flat = b * V + col
                blk, off = flat // BLK, flat % BLK
                if blk not in rows:
                    rows[blk] = np.zeros(BLK, np.float32)
                rows[blk][off] += position_boost[j]
        bidx = np.full((BCAP, 1), 1 << 20, np.int32)
        brows = np.zeros((BCAP, BLK), np.float32)
        for i, (blk, row) in enumerate(sorted(rows.items())):
            bidx[i, 0] = blk
            brows[i] = row
        m = dict(shared)
        m["locst"] = np.ascontiguousarray(lc.astype(np.int32).T)       # [S, BPC]
        m["userst"] = np.ascontiguousarray(
            np.repeat(uc.astype(np.int32)[None, :], P, axis=0))        # [P, BPC]
        m["bidx"] = bidx
        m["brows"] = brows
        in_maps.append(m)
    return in_maps, with_ln1_bias, with_bout


def kernel(**inputs) -> np.ndarray:
    in_maps, with_ln1_bias, with_bout = _prep_inputs(inputs)
    nc = _get_nc(with_ln1_bias, with_bout)
    res = bass_utils.run_bass_kernel_spmd(nc, in_maps, core_ids=list(range(NCORES)))
    return np.concatenate([r["out"] for r in res.results], axis=0)
